# revision 6
# baseline (speedup 1.0000x reference)
"""BCOP (block-convolution orthogonal parameterization) forward on 8 TRN2 cores.

Math (validated vs reference in fp32 numpy):
  - power iteration, unnormalized: v1 = A^T u0, v_{t+1} = (A^T A) v_t;
    s = sqrt((v10.v11)/(v10.v10)) equals the reference's u^T A v.
  - W0 = A/s; 20 Bjorck iters maintaining W and WT = W^T:
      G = W^T W  (lhsT=W,rhs=W);  M = 1.5 I - 0.5 G  (symmetric)
      W' = W M   (lhsT=WT,rhs=M); WT' = M WT (lhsT=M,rhs=WT)
  - downstream needs only WT = ortho^T:
      Z^T_i = WT[i+1] with rows>=128 zeroed -> PQ_i = matmul(lhsT=Z^T,rhs=Z^T)
      b1T[i1,j1] = (block_orth(PQ0,PQ1)[i1,j1])^T and b2 = block_orth(PQ2,PQ3),
      all products of symmetric matrices -> no transposes needed.
      p3[i,j] = sum b1[i1,j1] @ b2[i-i1,j-j1] = matmul(lhsT=b1T[..], rhs=b2[..])
      p_e[i,j] = H @ p3[i,j] = matmul(lhsT=WT[0], rhs=p3[i,j])
  - conv tap (kh,kw) uses stationary lhsT[ci,co] = p_e[kw,kh]; x circularly
    padded to 66x66 in SBUF; 18 accumulating matmuls (9 taps x 2 ci-tiles)
    per [128co x 512px] PSUM tile; bias added on the PSUM->SBUF evacuation.

Sharding: data-parallel over batch, 4 images per core; params + weight
construction replicated on all 8 cores; no collectives.
"""

import numpy as np

import concourse.bass as bass
import concourse.mybir as mybir
import concourse.tile as tile
from concourse import bacc
from concourse.bass_utils import run_bass_kernel_spmd
from concourse.masks import make_identity

P = 128
C = 256
NK = 5
N_CORES = 8
B_TOTAL = 32
B_CORE = B_TOTAL // N_CORES
H = 64
PH = 66
NPIX = H * H
POWER_ITERS = 10
BJORCK_ITERS = 20

F32 = mybir.dt.float32
F32R = mybir.dt.float32r
ALU = mybir.AluOpType
ACTF = mybir.ActivationFunctionType


def build_body(tc, out_ap, xs, pm, u0, bias_ap, ctx):
    nc = tc.nc

    persist = ctx.enter_context(tc.tile_pool(name="persist", bufs=1))
    small = ctx.enter_context(tc.tile_pool(name="small", bufs=3))

    # ---- constants ----
    ID1 = persist.tile([P, P], F32)
    make_identity(nc, ID1)
    I15 = persist.tile([P, 2, C], F32)
    I10 = persist.tile([P, 2, C], F32)
    nc.gpsimd.memset(I15[:], 0.0)
    nc.gpsimd.memset(I10[:], 0.0)
    for mt in range(2):
        # fill where x - y + mt*128 == 0  (diag of the mt-th 128-col block)
        nc.gpsimd.affine_select(
            out=I15[:, mt, :], in_=I15[:, mt, :], compare_op=ALU.not_equal,
            fill=1.5, base=mt * P, pattern=[[-1, C]], channel_multiplier=1)
        nc.gpsimd.affine_select(
            out=I10[:, mt, :], in_=I10[:, mt, :], compare_op=ALU.not_equal,
            fill=1.0, base=mt * P, pattern=[[-1, C]], channel_multiplier=1)

    bias_sb = persist.tile([P, 2, 1], F32)
    for mt in range(2):
        nc.sync.dma_start(bias_sb[:, mt, :], bias_ap[mt * P:(mt + 1) * P].unsqueeze(1))

    U_sb = persist.tile([P, NK, 2, 1], F32R)
    for k in range(NK):
        for tr in range(2):
            nc.sync.dma_start(U_sb[:, k, tr, :], u0[k, tr * P:(tr + 1) * P, :])

    RB = persist.tile([P, NK], F32)       # broadcast 1/s per matrix
    Wc_sb = persist.tile([P, 9, 2, C], F32R)  # final conv lhsT tiles, slot (i,j)

    with tc.tile_pool(name="build", bufs=1) as build, \
         tc.tile_pool(name="wstate", bufs=2) as wpool, \
         tc.tile_pool(name="mpool", bufs=4) as mpool, \
         tc.tile_pool(name="vpool", bufs=3) as vpool:

        Wcur = wpool.tile([P, NK, 2, C], F32R, tag="W")
        WTcur = wpool.tile([P, NK, 2, C], F32R, tag="WT")
        G_sb = build.tile([P, NK, 2, C], F32R, tag="big")  # slot shared with p3

        for k in range(NK):
            for tr in range(2):
                nc.sync.dma_start(Wcur[:, k, tr, :], pm[k, tr * P:(tr + 1) * P, :])

        # ================= phase 1: power iteration =================
        with tc.tile_pool(name="ps1", bufs=2, space="PSUM") as ps1:
            for k in range(NK):
                # G = A^T A
                for mt in range(2):
                    gps = ps1.tile([P, C], F32, tag="g")
                    for tr in range(2):
                        nc.tensor.matmul(
                            gps[:], (Wcur[:, k, tr, mt * P:(mt + 1) * P]),
                            (Wcur[:, k, tr, :]), start=(tr == 0), stop=(tr == 1))
                    nc.scalar.copy(G_sb[:, k, mt, :], gps[:])
                # v1 = A^T u0  (plain fp32: tiny)
                vcur = vpool.tile([P, 2], F32R, tag="v")
                for mt in range(2):
                    vps = ps1.tile([P, 1], F32, tag="vps")
                    for tr in range(2):
                        nc.tensor.matmul(
                            vps[:], Wcur[:, k, tr, mt * P:(mt + 1) * P].bitcast(F32),
                            U_sb[:, k, tr, :].bitcast(F32),
                            start=(tr == 0), stop=(tr == 1))
                    nc.scalar.copy(vcur[:, mt:mt + 1], vps[:])
                # v_{t+1} = G v_t, ten applications; v10 after 9, v11 after 10
                v10 = None
                for step in range(POWER_ITERS):
                    vnxt = vpool.tile([P, 2], F32R, tag="v", name=f"v_{k}_{step}")
                    for mt in range(2):
                        vps = ps1.tile([P, 1], F32, tag="vps")
                        for tr in range(2):
                            nc.tensor.matmul(
                                vps[:], G_sb[:, k, tr, mt * P:(mt + 1) * P].bitcast(F32),
                                vcur[:, tr:tr + 1].bitcast(F32),
                                start=(tr == 0), stop=(tr == 1))
                        nc.scalar.copy(vnxt[:, mt:mt + 1], vps[:])
                    if step == POWER_ITERS - 2:
                        v10 = vnxt
                    vcur = vnxt
                v11 = vcur
                assert v10 is not None
                # d0 = v10.v10 ; d1 = v10.v11 ; r = sqrt(d0/d1) = 1/s
                dps0 = ps1.tile([1, 1], F32, tag="dot")
                for tr in range(2):
                    nc.tensor.matmul(dps0[:], v10[:, tr:tr + 1].bitcast(F32),
                                     v10[:, tr:tr + 1].bitcast(F32),
                                     start=(tr == 0), stop=(tr == 1))
                dps1 = ps1.tile([1, 1], F32, tag="dot")
                for tr in range(2):
                    nc.tensor.matmul(dps1[:], v10[:, tr:tr + 1].bitcast(F32),
                                     v11[:, tr:tr + 1].bitcast(F32),
                                     start=(tr == 0), stop=(tr == 1))
                dsb = small.tile([1, 3], F32, tag="dsb")
                nc.vector.tensor_copy(dsb[:, 0:1], dps0[:])
                nc.vector.reciprocal(dsb[:, 1:2], dps1[:])
                nc.vector.tensor_mul(dsb[:, 2:3], dsb[:, 0:1], dsb[:, 1:2])
                rsb = small.tile([1, 1], F32, tag="rsb")
                nc.scalar.sqrt(rsb[:], dsb[:, 2:3])
                nc.gpsimd.partition_broadcast(RB[:, k:k + 1], rsb[:])
                # W0 = A * r (in place), then WT0 = W0^T via PE transpose
                nc.vector.tensor_scalar_mul(Wcur[:, k], Wcur[:, k], RB[:, k:k + 1])
                for tr in range(2):
                    for mt in range(2):
                        tps = ps1.tile([P, P], F32, tag="tp")
                        nc.tensor.transpose(
                            tps[:], Wcur[:, k, tr, mt * P:(mt + 1) * P].bitcast(F32),
                            ID1[:])
                        nc.scalar.copy(WTcur[:, k, mt, tr * P:(tr + 1) * P], tps[:])

        # ================= phase 2: Bjorck =================
        with tc.tile_pool(name="ps2", bufs=2, space="PSUM") as ps2:
            for it in range(BJORCK_ITERS):
                last = it == BJORCK_ITERS - 1
                Wnxt = None if last else wpool.tile([P, NK, 2, C], F32R, tag="W",
                                                    name=f"W_{it}")
                WTnxt = wpool.tile([P, NK, 2, C], F32R, tag="WT", name=f"WT_{it}")
                for k in range(NK):
                    Ms = []
                    for mt in range(2):
                        gps = ps2.tile([P, C], F32, tag="g")
                        for tr in range(2):
                            nc.tensor.matmul(
                                gps[:], (Wcur[:, k, tr, mt * P:(mt + 1) * P]),
                                (Wcur[:, k, tr, :]), start=(tr == 0), stop=(tr == 1))
                        m_sb = mpool.tile([P, C], F32R, tag="m", name=f"m_{it}_{k}_{mt}")
                        nc.vector.scalar_tensor_tensor(
                            m_sb[:], gps[:], -0.5, I15[:, mt, :],
                            op0=ALU.mult, op1=ALU.add)
                        Ms.append(m_sb)
                    for mt in range(2):
                        if not last:
                            wps = ps2.tile([P, C], F32, tag="w")
                            for tr in range(2):
                                nc.tensor.matmul(
                                    wps[:], (WTcur[:, k, tr, mt * P:(mt + 1) * P]),
                                    (Ms[tr][:]), start=(tr == 0), stop=(tr == 1))
                            nc.scalar.copy(Wnxt[:, k, mt, :], wps[:])
                        wtps = ps2.tile([P, C], F32, tag="wt")
                        for tr in range(2):
                            nc.tensor.matmul(
                                wtps[:], (Ms[tr][:, mt * P:(mt + 1) * P]),
                                (WTcur[:, k, tr, :]), start=(tr == 0), stop=(tr == 1))
                        nc.scalar.copy(WTnxt[:, k, mt, :], wtps[:])
                if Wnxt is not None:
                    Wcur = Wnxt
                WTcur = WTnxt
        WTfin = WTcur

        # ================= phase 3: weight assembly =================
        PQ_sb = build.tile([P, 4, 2, C], F32R)
        IP_sb = build.tile([P, 4, 2, C], F32R)
        b1T_sb = build.tile([P, 2, 2, 2, C], F32R)
        b2_sb = build.tile([P, 2, 2, 2, C], F32R)
        p3_sb = build.tile([P, 9, 2, C], F32R, tag="big")  # reuses G_sb's slot

        with tc.tile_pool(name="ps3", bufs=4, space="PSUM") as ps3:
            # PQ_i = Z Z^T (contract only first 128 rows of WT[i+1]) ; IP = I - PQ
            for i in range(4):
                for mt in range(2):
                    qps = ps3.tile([P, C], F32, tag="as")
                    nc.tensor.matmul(
                        qps[:], (WTfin[:, i + 1, 0, mt * P:(mt + 1) * P]),
                        (WTfin[:, i + 1, 0, :]), start=True, stop=True)
                    nc.scalar.copy(PQ_sb[:, i, mt, :], qps[:])
                    nc.vector.tensor_sub(IP_sb[:, i, mt, :], I10[:, mt, :],
                                         PQ_sb[:, i, mt, :])

            # b1T[i1,j1] = (j1? I-P1 : P1-swap...) see mock: X=(j1?IP1:PQ1), Y=(i1?IP0:PQ0)
            for i1 in range(2):
                for j1 in range(2):
                    Xb = IP_sb[:, 1] if j1 else PQ_sb[:, 1]
                    Yb = IP_sb[:, 0] if i1 else PQ_sb[:, 0]
                    for mt in range(2):
                        bps = ps3.tile([P, C], F32, tag="as")
                        for tr in range(2):
                            nc.tensor.matmul(
                                bps[:], (Xb[:, tr, mt * P:(mt + 1) * P]),
                                (Yb[:, tr, :]), start=(tr == 0), stop=(tr == 1))
                        nc.scalar.copy(b1T_sb[:, i1, j1, mt, :], bps[:])
            # b2[i2,j2] = (i2? I-P2 : P2) @ (j2? I-P3 : P3)
            for i2 in range(2):
                for j2 in range(2):
                    Xb = IP_sb[:, 2] if i2 else PQ_sb[:, 2]
                    Yb = IP_sb[:, 3] if j2 else PQ_sb[:, 3]
                    for mt in range(2):
                        bps = ps3.tile([P, C], F32, tag="as")
                        for tr in range(2):
                            nc.tensor.matmul(
                                bps[:], (Xb[:, tr, mt * P:(mt + 1) * P]),
                                (Yb[:, tr, :]), start=(tr == 0), stop=(tr == 1))
                        nc.scalar.copy(b2_sb[:, i2, j2, mt, :], bps[:])

            # p3[i,j] = sum_{i1,j1} b1[i1,j1] @ b2[i-i1,j-j1]
            for i in range(3):
                for j in range(3):
                    terms = [(i1, j1) for i1 in range(2) for j1 in range(2)
                             if 0 <= i - i1 < 2 and 0 <= j - j1 < 2]
                    for mt in range(2):
                        pps = ps3.tile([P, C], F32, tag="as")
                        n = len(terms) * 2
                        idx = 0
                        for (i1, j1) in terms:
                            for tr in range(2):
                                nc.tensor.matmul(
                                    pps[:],
                                    (b1T_sb[:, i1, j1, tr, mt * P:(mt + 1) * P]),
                                    (b2_sb[:, i - i1, j - j1, tr, :]),
                                    start=(idx == 0), stop=(idx == n - 1))
                                idx += 1
                        nc.scalar.copy(p3_sb[:, 3 * i + j, mt, :], pps[:])

            # p_e[i,j] = H @ p3[i,j]  (lhsT = H^T = WTfin[0])
            for i in range(3):
                for j in range(3):
                    for mt in range(2):
                        eps_ = ps3.tile([P, C], F32, tag="as")
                        for tr in range(2):
                            nc.tensor.matmul(
                                eps_[:], (WTfin[:, 0, tr, mt * P:(mt + 1) * P]),
                                (p3_sb[:, 3 * i + j, tr, :]),
                                start=(tr == 0), stop=(tr == 1))
                        nc.scalar.copy(Wc_sb[:, 3 * i + j, mt, :], eps_[:])

    # ================= phase 4: conv =================
    with tc.tile_pool(name="xpool", bufs=4) as xpool, \
         tc.tile_pool(name="opool", bufs=2) as opool, \
         tc.tile_pool(name="psC", bufs=6, space="PSUM") as psC:
        for b in range(B_CORE):
            xp = []
            for tr in range(2):
                xpt = xpool.tile([P, PH, PH], F32R, tag="xp", name=f"xp_{b}_{tr}")
                nc.sync.dma_start(xpt[:, 1:65, 1:65], xs[b, tr * P:(tr + 1) * P, :, :])
                # circular halo: columns first (corners propagate via rows)
                nc.vector.tensor_copy(xpt[:, 1:65, 0:1], xpt[:, 1:65, 64:65])
                nc.vector.tensor_copy(xpt[:, 1:65, 65:66], xpt[:, 1:65, 1:2])
                nc.vector.tensor_copy(xpt[:, 0:1, :], xpt[:, 64:65, :])
                nc.vector.tensor_copy(xpt[:, 65:66, :], xpt[:, 1:2, :])
                xp.append(xpt)
            for mt in range(2):
                osb = opool.tile([P, NPIX], F32, tag="osb", name=f"osb_{b}_{mt}")
                for pb in range(8):
                    ops = psC.tile([P, 512], F32, tag="o")
                    idx = 0
                    for kh in range(3):
                        for kw in range(3):
                            for tr in range(2):
                                nc.tensor.matmul(
                                    ops[:],
                                    (Wc_sb[:, 3 * kw + kh, tr, mt * P:(mt + 1) * P]),
                                    (xp[tr][:, pb * 8 + kh:pb * 8 + kh + 8,
                                              kw:kw + 64]),
                                    start=(idx == 0), stop=(idx == 17))
                                idx += 1
                    nc.scalar.activation(
                        osb[:, pb * 512:(pb + 1) * 512], ops[:], ACTF.Identity,
                        bias=bias_sb[:, mt, :], scale=1.0)
                nc.sync.dma_start(
                    out_ap[b, mt * P:(mt + 1) * P, :, :],
                    osb.rearrange("p (h w) -> p h w", w=H))


def build_program():
    from contextlib import ExitStack
    nc = bacc.Bacc("TRN2", target_bir_lowering=False, debug=False,
                   enable_asserts=False, num_devices=N_CORES)
    xs = nc.dram_tensor("xs", [B_CORE, C, H, H], F32R, kind="ExternalInput").ap()
    pm = nc.dram_tensor("pm", [NK, C, C], F32R, kind="ExternalInput").ap()
    u0 = nc.dram_tensor("u0", [NK, C, 1], F32R, kind="ExternalInput").ap()
    bias = nc.dram_tensor("bias", [C], F32, kind="ExternalInput").ap()
    out = nc.dram_tensor("out", [B_CORE, C, H, H], F32, kind="ExternalOutput").ap()
    with tile.TileContext(nc) as tc:
        with ExitStack() as ctx:
            build_body(tc, out, xs, pm, u0, bias, ctx)
    nc.compile()
    return nc


_cached_nc = None


def kernel(x, param_matrices, init_u, bias):
    global _cached_nc
    if _cached_nc is None:
        _cached_nc = build_program()
    nc = _cached_nc
    x = np.ascontiguousarray(np.asarray(x, dtype=np.float32))
    pm = np.ascontiguousarray(np.asarray(param_matrices, dtype=np.float32))
    u0 = np.ascontiguousarray(np.asarray(init_u, dtype=np.float32))
    b = np.ascontiguousarray(np.asarray(bias, dtype=np.float32))
    in_maps = [
        {"xs": x[i * B_CORE:(i + 1) * B_CORE], "pm": pm, "u0": u0, "bias": b}
        for i in range(N_CORES)
    ]
    res = run_bass_kernel_spmd(nc, in_maps, core_ids=list(range(N_CORES)))
    return np.concatenate([r["out"] for r in res.results], axis=0)


if __name__ == "__main__":
    import reference
    inputs = {k: np.asarray(v) for k, v in reference.setup_inputs().items()}
    out = kernel(**inputs)
    print(out.shape, out.dtype)


# revision 8
# speedup vs baseline: 1.0050x; 1.0050x over previous
"""BCOP (block-convolution orthogonal parameterization) forward on 8 TRN2 cores.

Math (validated vs reference in fp32 numpy):
  - power iteration, unnormalized: v1 = A^T u0, v_{t+1} = (A^T A) v_t;
    s = sqrt((v10.v11)/(v10.v10)) equals the reference's u^T A v.
  - W0 = A/s; 20 Bjorck iters maintaining W and WT = W^T:
      G = W^T W  (lhsT=W,rhs=W);  M = 1.5 I - 0.5 G  (symmetric)
      W' = W M   (lhsT=WT,rhs=M); WT' = M WT (lhsT=M,rhs=WT)
  - downstream needs only WT = ortho^T:
      Z^T_i = WT[i+1] with rows>=128 zeroed -> PQ_i = matmul(lhsT=Z^T,rhs=Z^T)
      b1T[i1,j1] = (block_orth(PQ0,PQ1)[i1,j1])^T and b2 = block_orth(PQ2,PQ3),
      all products of symmetric matrices -> no transposes needed.
      p3[i,j] = sum b1[i1,j1] @ b2[i-i1,j-j1] = matmul(lhsT=b1T[..], rhs=b2[..])
      p_e[i,j] = H @ p3[i,j] = matmul(lhsT=WT[0], rhs=p3[i,j])
  - conv tap (kh,kw) uses stationary lhsT[ci,co] = p_e[kw,kh]; x circularly
    padded to 66x66 in SBUF; 18 accumulating matmuls (9 taps x 2 ci-tiles)
    per [128co x 512px] PSUM tile; bias added on the PSUM->SBUF evacuation.

Sharding: data-parallel over batch, 4 images per core; params + weight
construction replicated on all 8 cores; no collectives.
"""

import numpy as np

import concourse.bass as bass
import concourse.mybir as mybir
import concourse.tile as tile
from concourse import bacc
from concourse.bass_utils import run_bass_kernel_spmd
from concourse.masks import make_identity

P = 128
C = 256
NK = 5
N_CORES = 8
B_TOTAL = 32
B_CORE = B_TOTAL // N_CORES
H = 64
PH = 66
NPIX = H * H
POWER_ITERS = 10
BJORCK_ITERS = 20

F32 = mybir.dt.float32
F32R = mybir.dt.float32r
ALU = mybir.AluOpType
ACTF = mybir.ActivationFunctionType


def build_body(tc, out_ap, xs, pm, u0, bias_ap, ctx):
    nc = tc.nc

    persist = ctx.enter_context(tc.tile_pool(name="persist", bufs=1))
    small = ctx.enter_context(tc.tile_pool(name="small", bufs=3))

    U_sb = persist.tile([P, NK, 2, 1], F32R)
    for k in range(NK):
        for tr in range(2):
            nc.sync.dma_start(U_sb[:, k, tr, :], u0[k, tr * P:(tr + 1) * P, :])
    bias_sb = persist.tile([P, 2, 1], F32)
    for mt in range(2):
        nc.sync.dma_start(bias_sb[:, mt, :], bias_ap[mt * P:(mt + 1) * P].unsqueeze(1))

    # ---- constants (DVE/ACT to keep startup off the slow path) ----
    ID1 = persist.tile([P, P], F32)
    make_identity(nc, ID1)
    I15 = persist.tile([P, 2, C], F32)
    I10 = persist.tile([P, 2, C], F32)
    nc.vector.memset(I15[:], 0.0)
    nc.vector.memset(I10[:], 0.0)
    for mt in range(2):
        nc.scalar.mul(I15[:, mt, mt * P:(mt + 1) * P], ID1[:], 1.5)
        nc.scalar.mul(I10[:, mt, mt * P:(mt + 1) * P], ID1[:], 1.0)

    RB = persist.tile([P, NK], F32)       # broadcast 1/s per matrix
    Wc_sb = persist.tile([P, 9, 2, C], F32R)  # final conv lhsT tiles, slot (i,j)

    with tc.tile_pool(name="build", bufs=1) as build, \
         tc.tile_pool(name="wstate", bufs=2) as wpool, \
         tc.tile_pool(name="mpool", bufs=4) as mpool, \
         tc.tile_pool(name="vpool", bufs=3) as vpool:

        Wcur = wpool.tile([P, NK, 2, C], F32R, tag="W")
        WTcur = wpool.tile([P, NK, 2, C], F32R, tag="WT")
        G_sb = build.tile([P, NK, 2, C], F32R, tag="big")  # slot shared with p3

        for k in range(NK):
            for tr in range(2):
                nc.sync.dma_start(Wcur[:, k, tr, :], pm[k, tr * P:(tr + 1) * P, :])

        # ================= phase 1: power iteration =================
        with tc.tile_pool(name="ps1", bufs=2, space="PSUM") as ps1:
            for k in range(NK):
                # G = A^T A
                for mt in range(2):
                    gps = ps1.tile([P, C], F32, tag="g")
                    for tr in range(2):
                        nc.tensor.matmul(
                            gps[:], (Wcur[:, k, tr, mt * P:(mt + 1) * P]),
                            (Wcur[:, k, tr, :]), start=(tr == 0), stop=(tr == 1))
                    nc.scalar.copy(G_sb[:, k, mt, :], gps[:])
                # v1 = A^T u0  (plain fp32: tiny)
                vcur = vpool.tile([P, 2], F32R, tag="v")
                for mt in range(2):
                    vps = ps1.tile([P, 1], F32, tag="vps", bufs=4)
                    for tr in range(2):
                        nc.tensor.matmul(
                            vps[:], Wcur[:, k, tr, mt * P:(mt + 1) * P].bitcast(F32),
                            U_sb[:, k, tr, :].bitcast(F32),
                            start=(tr == 0), stop=(tr == 1))
                    nc.scalar.copy(vcur[:, mt:mt + 1], vps[:])
                # v_{t+1} = G v_t, ten applications; v10 after 9, v11 after 10
                v10 = None
                for step in range(POWER_ITERS):
                    vnxt = vpool.tile([P, 2], F32R, tag="v", name=f"v_{k}_{step}")
                    for mt in range(2):
                        vps = ps1.tile([P, 1], F32, tag="vps", bufs=4)
                        for tr in range(2):
                            nc.tensor.matmul(
                                vps[:], G_sb[:, k, tr, mt * P:(mt + 1) * P].bitcast(F32),
                                vcur[:, tr:tr + 1].bitcast(F32),
                                start=(tr == 0), stop=(tr == 1))
                        nc.scalar.copy(vnxt[:, mt:mt + 1], vps[:])
                    if step == POWER_ITERS - 2:
                        v10 = vnxt
                    vcur = vnxt
                v11 = vcur
                assert v10 is not None
                # d0 = v10.v10 ; d1 = v10.v11 ; r = sqrt(d0/d1) = 1/s
                dps0 = ps1.tile([1, 1], F32, tag="vps", bufs=4, name="dps0")
                for tr in range(2):
                    nc.tensor.matmul(dps0[:], v10[:, tr:tr + 1].bitcast(F32),
                                     v10[:, tr:tr + 1].bitcast(F32),
                                     start=(tr == 0), stop=(tr == 1))
                dps1 = ps1.tile([1, 1], F32, tag="vps", bufs=4, name="dps1")
                for tr in range(2):
                    nc.tensor.matmul(dps1[:], v10[:, tr:tr + 1].bitcast(F32),
                                     v11[:, tr:tr + 1].bitcast(F32),
                                     start=(tr == 0), stop=(tr == 1))
                dsb = small.tile([1, 3], F32, tag="dsb")
                nc.vector.tensor_copy(dsb[:, 0:1], dps0[:])
                nc.vector.reciprocal(dsb[:, 1:2], dps1[:])
                nc.vector.tensor_mul(dsb[:, 2:3], dsb[:, 0:1], dsb[:, 1:2])
                rsb = small.tile([1, 1], F32, tag="rsb")
                nc.scalar.sqrt(rsb[:], dsb[:, 2:3])
                nc.gpsimd.partition_broadcast(RB[:, k:k + 1], rsb[:])
                # W0 = A * r (in place), then WT0 = W0^T via PE transpose
                nc.vector.tensor_scalar_mul(Wcur[:, k], Wcur[:, k], RB[:, k:k + 1])
                for tr in range(2):
                    for mt in range(2):
                        tps = ps1.tile([P, P], F32, tag="tp")
                        nc.tensor.transpose(
                            tps[:], Wcur[:, k, tr, mt * P:(mt + 1) * P].bitcast(F32),
                            ID1[:])
                        nc.scalar.copy(WTcur[:, k, mt, tr * P:(tr + 1) * P], tps[:])

        # ================= phase 2: Bjorck =================
        with tc.tile_pool(name="ps2", bufs=2, space="PSUM") as ps2:
            for it in range(BJORCK_ITERS):
                last = it == BJORCK_ITERS - 1
                Wnxt = None if last else wpool.tile([P, NK, 2, C], F32R, tag="W",
                                                    name=f"W_{it}")
                WTnxt = wpool.tile([P, NK, 2, C], F32R, tag="WT", name=f"WT_{it}")
                for k in range(NK):
                    Ms = []
                    for mt in range(2):
                        gps = ps2.tile([P, C], F32, tag="g")
                        for tr in range(2):
                            nc.tensor.matmul(
                                gps[:], (Wcur[:, k, tr, mt * P:(mt + 1) * P]),
                                (Wcur[:, k, tr, :]), start=(tr == 0), stop=(tr == 1))
                        m_sb = mpool.tile([P, C], F32R, tag="m", name=f"m_{it}_{k}_{mt}")
                        nc.vector.scalar_tensor_tensor(
                            m_sb[:], gps[:], -0.5, I15[:, mt, :],
                            op0=ALU.mult, op1=ALU.add)
                        Ms.append(m_sb)
                    for mt in range(2):
                        if not last:
                            wps = ps2.tile([P, C], F32, tag="w")
                            for tr in range(2):
                                nc.tensor.matmul(
                                    wps[:], (WTcur[:, k, tr, mt * P:(mt + 1) * P]),
                                    (Ms[tr][:]), start=(tr == 0), stop=(tr == 1))
                            nc.scalar.copy(Wnxt[:, k, mt, :], wps[:])
                        wtps = ps2.tile([P, C], F32, tag="wt")
                        for tr in range(2):
                            nc.tensor.matmul(
                                wtps[:], (Ms[tr][:, mt * P:(mt + 1) * P]),
                                (WTcur[:, k, tr, :]), start=(tr == 0), stop=(tr == 1))
                        nc.scalar.copy(WTnxt[:, k, mt, :], wtps[:])
                if Wnxt is not None:
                    Wcur = Wnxt
                WTcur = WTnxt
        WTfin = WTcur

        # ================= phase 3: weight assembly =================
        PQ_sb = build.tile([P, 4, 2, C], F32R)
        IP_sb = build.tile([P, 4, 2, C], F32R)
        b1T_sb = build.tile([P, 2, 2, 2, C], F32R)
        b2_sb = build.tile([P, 2, 2, 2, C], F32R)
        p3_sb = build.tile([P, 9, 2, C], F32R, tag="big")  # reuses G_sb's slot

        with tc.tile_pool(name="ps3", bufs=4, space="PSUM") as ps3:
            # PQ_i = Z Z^T (contract only first 128 rows of WT[i+1]) ; IP = I - PQ
            for i in range(4):
                for mt in range(2):
                    qps = ps3.tile([P, C], F32, tag="as")
                    nc.tensor.matmul(
                        qps[:], (WTfin[:, i + 1, 0, mt * P:(mt + 1) * P]),
                        (WTfin[:, i + 1, 0, :]), start=True, stop=True)
                    nc.scalar.copy(PQ_sb[:, i, mt, :], qps[:])
                    nc.vector.tensor_sub(IP_sb[:, i, mt, :], I10[:, mt, :],
                                         PQ_sb[:, i, mt, :])

            # b1T[i1,j1] = (j1? I-P1 : P1-swap...) see mock: X=(j1?IP1:PQ1), Y=(i1?IP0:PQ0)
            for i1 in range(2):
                for j1 in range(2):
                    Xb = IP_sb[:, 1] if j1 else PQ_sb[:, 1]
                    Yb = IP_sb[:, 0] if i1 else PQ_sb[:, 0]
                    for mt in range(2):
                        bps = ps3.tile([P, C], F32, tag="as")
                        for tr in range(2):
                            nc.tensor.matmul(
                                bps[:], (Xb[:, tr, mt * P:(mt + 1) * P]),
                                (Yb[:, tr, :]), start=(tr == 0), stop=(tr == 1))
                        nc.scalar.copy(b1T_sb[:, i1, j1, mt, :], bps[:])
            # b2[i2,j2] = (i2? I-P2 : P2) @ (j2? I-P3 : P3)
            for i2 in range(2):
                for j2 in range(2):
                    Xb = IP_sb[:, 2] if i2 else PQ_sb[:, 2]
                    Yb = IP_sb[:, 3] if j2 else PQ_sb[:, 3]
                    for mt in range(2):
                        bps = ps3.tile([P, C], F32, tag="as")
                        for tr in range(2):
                            nc.tensor.matmul(
                                bps[:], (Xb[:, tr, mt * P:(mt + 1) * P]),
                                (Yb[:, tr, :]), start=(tr == 0), stop=(tr == 1))
                        nc.scalar.copy(b2_sb[:, i2, j2, mt, :], bps[:])

            # p3[i,j] = sum_{i1,j1} b1[i1,j1] @ b2[i-i1,j-j1]
            for i in range(3):
                for j in range(3):
                    terms = [(i1, j1) for i1 in range(2) for j1 in range(2)
                             if 0 <= i - i1 < 2 and 0 <= j - j1 < 2]
                    for mt in range(2):
                        pps = ps3.tile([P, C], F32, tag="as")
                        n = len(terms) * 2
                        idx = 0
                        for (i1, j1) in terms:
                            for tr in range(2):
                                nc.tensor.matmul(
                                    pps[:],
                                    (b1T_sb[:, i1, j1, tr, mt * P:(mt + 1) * P]),
                                    (b2_sb[:, i - i1, j - j1, tr, :]),
                                    start=(idx == 0), stop=(idx == n - 1))
                                idx += 1
                        nc.scalar.copy(p3_sb[:, 3 * i + j, mt, :], pps[:])

            # p_e[i,j] = H @ p3[i,j]  (lhsT = H^T = WTfin[0])
            for i in range(3):
                for j in range(3):
                    for mt in range(2):
                        eps_ = ps3.tile([P, C], F32, tag="as")
                        for tr in range(2):
                            nc.tensor.matmul(
                                eps_[:], (WTfin[:, 0, tr, mt * P:(mt + 1) * P]),
                                (p3_sb[:, 3 * i + j, tr, :]),
                                start=(tr == 0), stop=(tr == 1))
                        nc.scalar.copy(Wc_sb[:, 3 * i + j, mt, :], eps_[:])

    # ================= phase 4: conv =================
    with tc.tile_pool(name="xpool", bufs=6) as xpool, \
         tc.tile_pool(name="opool", bufs=3) as opool, \
         tc.tile_pool(name="psC", bufs=6, space="PSUM") as psC:
        for b in range(B_CORE):
            xp = []
            for tr in range(2):
                xpt = xpool.tile([P, PH, PH], F32R, tag="xp", name=f"xp_{b}_{tr}")
                nc.sync.dma_start(xpt[:, 1:65, 1:65], xs[b, tr * P:(tr + 1) * P, :, :])
                # circular halo: columns first (corners propagate via rows)
                nc.vector.tensor_copy(xpt[:, 1:65, 0:1], xpt[:, 1:65, 64:65])
                nc.vector.tensor_copy(xpt[:, 1:65, 65:66], xpt[:, 1:65, 1:2])
                nc.vector.tensor_copy(xpt[:, 0:1, :], xpt[:, 64:65, :])
                nc.vector.tensor_copy(xpt[:, 65:66, :], xpt[:, 1:2, :])
                xp.append(xpt)
            for mt in range(2):
                osb = opool.tile([P, NPIX], F32, tag="osb", name=f"osb_{b}_{mt}")
                for pb in range(8):
                    ops = psC.tile([P, 512], F32, tag="o")
                    idx = 0
                    for kh in range(3):
                        for kw in range(3):
                            for tr in range(2):
                                nc.tensor.matmul(
                                    ops[:],
                                    (Wc_sb[:, 3 * kw + kh, tr, mt * P:(mt + 1) * P]),
                                    (xp[tr][:, pb * 8 + kh:pb * 8 + kh + 8,
                                              kw:kw + 64]),
                                    start=(idx == 0), stop=(idx == 17))
                                idx += 1
                    nc.scalar.activation(
                        osb[:, pb * 512:(pb + 1) * 512], ops[:], ACTF.Identity,
                        bias=bias_sb[:, mt, :], scale=1.0)
                    nc.sync.dma_start(
                        out_ap[b, mt * P:(mt + 1) * P, pb * 8:(pb + 1) * 8, :],
                        osb[:, pb * 512:(pb + 1) * 512].rearrange(
                            "p (h w) -> p h w", w=H))


def build_program():
    from contextlib import ExitStack
    nc = bacc.Bacc("TRN2", target_bir_lowering=False, debug=False,
                   enable_asserts=False, num_devices=N_CORES)
    xs = nc.dram_tensor("xs", [B_CORE, C, H, H], F32R, kind="ExternalInput").ap()
    pm = nc.dram_tensor("pm", [NK, C, C], F32R, kind="ExternalInput").ap()
    u0 = nc.dram_tensor("u0", [NK, C, 1], F32R, kind="ExternalInput").ap()
    bias = nc.dram_tensor("bias", [C], F32, kind="ExternalInput").ap()
    out = nc.dram_tensor("out", [B_CORE, C, H, H], F32, kind="ExternalOutput").ap()
    with tile.TileContext(nc) as tc:
        with ExitStack() as ctx:
            build_body(tc, out, xs, pm, u0, bias, ctx)
    nc.compile()
    return nc


_cached_nc = None


def kernel(x, param_matrices, init_u, bias):
    global _cached_nc
    if _cached_nc is None:
        _cached_nc = build_program()
    nc = _cached_nc
    x = np.ascontiguousarray(np.asarray(x, dtype=np.float32))
    pm = np.ascontiguousarray(np.asarray(param_matrices, dtype=np.float32))
    u0 = np.ascontiguousarray(np.asarray(init_u, dtype=np.float32))
    b = np.ascontiguousarray(np.asarray(bias, dtype=np.float32))
    in_maps = [
        {"xs": x[i * B_CORE:(i + 1) * B_CORE], "pm": pm, "u0": u0, "bias": b}
        for i in range(N_CORES)
    ]
    res = run_bass_kernel_spmd(nc, in_maps, core_ids=list(range(N_CORES)))
    return np.concatenate([r["out"] for r in res.results], axis=0)


if __name__ == "__main__":
    import reference
    inputs = {k: np.asarray(v) for k, v in reference.setup_inputs().items()}
    out = kernel(**inputs)
    print(out.shape, out.dtype)


# revision 9
# speedup vs baseline: 1.1100x; 1.1045x over previous
"""BCOP (block-convolution orthogonal parameterization) forward on 8 TRN2 cores.

Math (validated vs reference in fp32 numpy):
  - power iteration via repeated squaring: G = A^T A; with v1 = A^T u0,
    d0 = v1.(G^18 v1), d1 = v1.(G^19 v1) reproduce the reference's
    normalized-power-iteration sigma: s = sqrt(d1/d0); G^18 v1 = G16@(G2@v1).
  - W0 = A/s; 20 Bjorck iters maintaining W and WT = W^T:
      G = W^T W  (lhsT=W,rhs=W);  M = 1.5 I - 0.5 G  (symmetric)
      W' = W M   (lhsT=WT,rhs=M); WT' = M WT (lhsT=M,rhs=WT)
  - downstream needs only WT = ortho^T:
      Z^T_i = WT[i+1] with rows>=128 zeroed -> PQ_i = matmul(lhsT=Z^T,rhs=Z^T)
      b1T[i1,j1] = (block_orth(PQ0,PQ1)[i1,j1])^T and b2 = block_orth(PQ2,PQ3),
      all products of symmetric matrices -> no transposes needed.
      p3[i,j] = sum b1[i1,j1] @ b2[i-i1,j-j1] = matmul(lhsT=b1T[..], rhs=b2[..])
      p_e[i,j] = H @ p3[i,j] = matmul(lhsT=WT[0], rhs=p3[i,j])
  - conv tap (kh,kw) uses stationary lhsT[ci,co] = p_e[kw,kh]; x circularly
    padded to 66x66 in SBUF; 18 accumulating matmuls (9 taps x 2 ci-tiles)
    per [128co x 512px] PSUM tile; bias added on the PSUM->SBUF evacuation.

PSUM discipline: every accumulation group owns a whole bank; a [128,512] tile
holds both 128-row output halves of a 256x256 product as ONE group (start=True
on the first matmul clears the whole zero-region; the other half's first
touch then writes fresh per-element).

Sharding: data-parallel over batch, 4 images per core; params + weight
construction replicated on all 8 cores; no collectives.
"""

import numpy as np

import concourse.bass as bass
import concourse.mybir as mybir
import concourse.tile as tile
from concourse import bacc
from concourse.bass_utils import run_bass_kernel_spmd
from concourse.masks import make_identity

P = 128
C = 256
NK = 5
N_CORES = 8
B_TOTAL = 32
B_CORE = B_TOTAL // N_CORES
H = 64
PH = 66
NPIX = H * H
BJORCK_ITERS = 20

F32 = mybir.dt.float32
F32R = mybir.dt.float32r
ALU = mybir.AluOpType
ACTF = mybir.ActivationFunctionType


def build_body(tc, out_ap, xs, pm, u0, bias_ap, ctx):
    nc = tc.nc

    persist = ctx.enter_context(tc.tile_pool(name="persist", bufs=1))
    small = ctx.enter_context(tc.tile_pool(name="small", bufs=3))

    U_sb = persist.tile([P, NK, 2, 1], F32R)
    for k in range(NK):
        for tr in range(2):
            nc.sync.dma_start(U_sb[:, k, tr, :], u0[k, tr * P:(tr + 1) * P, :])
    bias_sb = persist.tile([P, 2, 1], F32)
    for mt in range(2):
        nc.sync.dma_start(bias_sb[:, mt, :], bias_ap[mt * P:(mt + 1) * P].unsqueeze(1))

    # ---- constants (DVE/ACT; gpsimd only for the small 128x128 identity) ----
    ID1 = persist.tile([P, P], F32)
    make_identity(nc, ID1)
    I15 = persist.tile([P, 2, C], F32)
    I10 = persist.tile([P, 2, C], F32)
    nc.vector.memset(I15[:], 0.0)
    nc.vector.memset(I10[:], 0.0)
    for mt in range(2):
        nc.scalar.mul(I15[:, mt, mt * P:(mt + 1) * P], ID1[:], 1.5)
        nc.scalar.mul(I10[:, mt, mt * P:(mt + 1) * P], ID1[:], 1.0)
    I15f = I15.rearrange("p a b -> p (a b)")

    RB = persist.tile([P, NK], F32)       # broadcast 1/s per matrix
    Wc_sb = persist.tile([P, 9, 2, C], F32R)  # final conv lhsT tiles, slot (i,j)

    def flat(ap3):
        """[P, 2, C] -> [P, 512]"""
        return ap3.rearrange("p a b -> p (a b)")

    def prod_mms(out_ps, X3, Y3, n_tr=2):
        """One 256x256 product dst = X^T-style: out[mt] += X[tr][:, mt]^T @
        Y[tr]; X3/Y3 are [P, 2, C] APs; ONE accumulation group per bank."""
        first = True
        for mt in range(2):
            for tr in range(n_tr):
                last = (mt == 1 and tr == n_tr - 1)
                nc.tensor.matmul(out_ps[:, mt * C:(mt + 1) * C],
                                 X3[:, tr, mt * P:(mt + 1) * P], Y3[:, tr, :],
                                 start=first, stop=last)
                first = False

    with tc.tile_pool(name="build", bufs=1) as build, \
         tc.tile_pool(name="wstate", bufs=2) as wpool, \
         tc.tile_pool(name="mpool", bufs=4) as mpool, \
         tc.tile_pool(name="vpool", bufs=8) as vpool:

        Wcur = wpool.tile([P, NK, 2, C], F32R, tag="W")
        WTcur = wpool.tile([P, NK, 2, C], F32R, tag="WT")
        G_sb = build.tile([P, NK, 2, C], F32R, tag="big")  # slot shared with p3
        G2_sb = build.tile([P, NK, 2, C], F32R)
        G16_sb = build.tile([P, NK, 2, C], F32R)

        for k in range(NK):
            for tr in range(2):
                nc.sync.dma_start(Wcur[:, k, tr, :], pm[k, tr * P:(tr + 1) * P, :])

        # ============ phase 1: sigma via repeated squaring ============
        with tc.tile_pool(name="ps1", bufs=2, space="PSUM") as ps1:
            for k in range(NK):
                # G = A^T A ; then G2 -> G4 -> G8 -> G16 by squaring
                gps = ps1.tile([P, 2 * C], F32, tag="sq")
                prod_mms(gps, Wcur[:, k], Wcur[:, k])
                nc.scalar.copy(flat(G_sb[:, k]), gps[:])

                prev = G_sb[:, k]
                for pw in (2, 4, 8, 16):
                    sq = ps1.tile([P, 2 * C], F32, tag="sq", name=f"sq{pw}_{k}")
                    prod_mms(sq, prev, prev)
                    if pw == 2:
                        dst = G2_sb[:, k]
                    elif pw == 16:
                        dst = G16_sb[:, k]
                    else:
                        dst = build.tile([P, 2, C], F32R, tag="gtmp",
                                         name=f"g{pw}_{k}", bufs=2)
                    if pw in (4, 16):
                        nc.vector.tensor_copy(flat(dst), sq[:])
                    else:
                        nc.scalar.copy(flat(dst), sq[:])
                    prev = dst

                # v1 = A^T u0 (plain fp32; tiny)
                def matvec(G3, vin, nm):
                    vout = vpool.tile([P, 2], F32R, tag="v", name=f"v_{nm}")
                    for mt in range(2):
                        vps = ps1.tile([P, 1], F32, tag="vps", bufs=4,
                                       name=f"vp_{nm}_{mt}")
                        for tr in range(2):
                            nc.tensor.matmul(
                                vps[:], G3[:, tr, mt * P:(mt + 1) * P].bitcast(F32),
                                vin[:, tr:tr + 1].bitcast(F32),
                                start=(tr == 0), stop=(tr == 1))
                        nc.scalar.copy(vout[:, mt:mt + 1], vps[:])
                    return vout

                v1 = matvec(Wcur[:, k], U_sb[:, k], f"v1_{k}")
                m1 = matvec(G2_sb[:, k], v1, f"m1_{k}")
                m2 = matvec(G16_sb[:, k], m1, f"m2_{k}")
                m3 = matvec(G_sb[:, k], m2, f"m3_{k}")

                # d0 = v1.m2 ; d1 = v1.m3 ; r = sqrt(d0/d1) = 1/s
                def dot(va, vb, nm):
                    dps = ps1.tile([1, 1], F32, tag="vps", bufs=4, name=f"d_{nm}")
                    for tr in range(2):
                        nc.tensor.matmul(dps[:], va[:, tr:tr + 1].bitcast(F32),
                                         vb[:, tr:tr + 1].bitcast(F32),
                                         start=(tr == 0), stop=(tr == 1))
                    return dps

                dps0 = dot(v1, m2, f"0_{k}")
                dps1 = dot(v1, m3, f"1_{k}")
                dsb = small.tile([1, 3], F32, tag="dsb")
                nc.vector.tensor_copy(dsb[:, 0:1], dps0[:])
                nc.vector.reciprocal(dsb[:, 1:2], dps1[:])
                nc.vector.tensor_mul(dsb[:, 2:3], dsb[:, 0:1], dsb[:, 1:2])
                rsb = small.tile([1, 1], F32, tag="rsb")
                nc.scalar.sqrt(rsb[:], dsb[:, 2:3])
                nc.gpsimd.partition_broadcast(RB[:, k:k + 1], rsb[:])
                # W0 = A * r (in place), then WT0 = W0^T via PE transpose
                nc.vector.tensor_scalar_mul(Wcur[:, k], Wcur[:, k], RB[:, k:k + 1])
                for tr in range(2):
                    for mt in range(2):
                        tps = ps1.tile([P, P], F32, tag="tp")
                        nc.tensor.transpose(
                            tps[:], Wcur[:, k, tr, mt * P:(mt + 1) * P].bitcast(F32),
                            ID1[:])
                        nc.scalar.copy(WTcur[:, k, mt, tr * P:(tr + 1) * P], tps[:])

        # ================= phase 2: Bjorck =================
        with tc.tile_pool(name="ps2", bufs=2, space="PSUM") as ps2:
            for it in range(BJORCK_ITERS):
                last = it == BJORCK_ITERS - 1
                Wnxt = None if last else wpool.tile([P, NK, 2, C], F32R, tag="W",
                                                    name=f"W_{it}")
                WTnxt = wpool.tile([P, NK, 2, C], F32R, tag="WT", name=f"WT_{it}")
                for k in range(NK):
                    gps = ps2.tile([P, 2 * C], F32, tag="g")
                    prod_mms(gps, Wcur[:, k], Wcur[:, k])
                    m_sb = mpool.tile([P, 2 * C], F32R, tag="m", name=f"m_{it}_{k}")
                    nc.vector.scalar_tensor_tensor(
                        m_sb[:], gps[:], -0.5, I15f, op0=ALU.mult, op1=ALU.add)
                    m3 = m_sb.rearrange("p (a b) -> p a b", b=C)
                    if not last:
                        wps = ps2.tile([P, 2 * C], F32, tag="w")
                        prod_mms(wps, WTcur[:, k], m3)
                        nc.scalar.copy(flat(Wnxt[:, k]), wps[:])
                    wtps = ps2.tile([P, 2 * C], F32, tag="wt")
                    prod_mms(wtps, m3, WTcur[:, k])
                    nc.vector.tensor_copy(flat(WTnxt[:, k]), wtps[:])
                if Wnxt is not None:
                    Wcur = Wnxt
                WTcur = WTnxt
        WTfin = WTcur

        # ================= phase 3: weight assembly =================
        PQ_sb = build.tile([P, 4, 2, C], F32R)
        IP_sb = build.tile([P, 4, 2, C], F32R)
        b1T_sb = build.tile([P, 2, 2, 2, C], F32R)
        b2_sb = build.tile([P, 2, 2, 2, C], F32R)
        p3_sb = build.tile([P, 9, 2, C], F32R, tag="big")  # reuses G_sb's slot

        with tc.tile_pool(name="ps3", bufs=4, space="PSUM") as ps3:
            # PQ_i = Z Z^T (contract only first 128 rows of WT[i+1]) ; IP = I-PQ
            for i in range(4):
                qps = ps3.tile([P, 2 * C], F32, tag="as", name=f"q_{i}")
                prod_mms(qps, WTfin[:, i + 1], WTfin[:, i + 1], n_tr=1)
                nc.scalar.copy(flat(PQ_sb[:, i]), qps[:])
                nc.vector.tensor_sub(flat(IP_sb[:, i]), flat(I10), flat(PQ_sb[:, i]))

            def symprod(dst3, X3, Y3, nm):
                bps = ps3.tile([P, 2 * C], F32, tag="as", name=f"bp_{nm}")
                prod_mms(bps, X3, Y3)
                nc.scalar.copy(flat(dst3), bps[:])

            # b1T[i1,j1] = (j1? IP1:PQ1) @ (i1? IP0:PQ0)
            for i1 in range(2):
                for j1 in range(2):
                    symprod(b1T_sb[:, i1, j1],
                            IP_sb[:, 1] if j1 else PQ_sb[:, 1],
                            IP_sb[:, 0] if i1 else PQ_sb[:, 0], f"b1T{i1}{j1}")
            # b2[i2,j2] = (i2? IP2:PQ2) @ (j2? IP3:PQ3)
            for i2 in range(2):
                for j2 in range(2):
                    symprod(b2_sb[:, i2, j2],
                            IP_sb[:, 2] if i2 else PQ_sb[:, 2],
                            IP_sb[:, 3] if j2 else PQ_sb[:, 3], f"b2{i2}{j2}")

            # p3[i,j] = sum_{i1,j1} b1[i1,j1] @ b2[i-i1,j-j1]
            for i in range(3):
                for j in range(3):
                    terms = [(i1, j1) for i1 in range(2) for j1 in range(2)
                             if 0 <= i - i1 < 2 and 0 <= j - j1 < 2]
                    pps = ps3.tile([P, 2 * C], F32, tag="as", name=f"p3_{i}{j}")
                    nmm = len(terms) * 4
                    idx = 0
                    for mt in range(2):
                        for (i1, j1) in terms:
                            for tr in range(2):
                                idx += 1
                                nc.tensor.matmul(
                                    pps[:, mt * C:(mt + 1) * C],
                                    b1T_sb[:, i1, j1, tr, mt * P:(mt + 1) * P],
                                    b2_sb[:, i - i1, j - j1, tr, :],
                                    start=(idx == 1), stop=(idx == nmm))
                    nc.vector.tensor_copy(flat(p3_sb[:, 3 * i + j]), pps[:])

            # p_e[i,j] = H @ p3[i,j]  (lhsT = H^T = WTfin[0])
            for i in range(3):
                for j in range(3):
                    eps_ = ps3.tile([P, 2 * C], F32, tag="as", name=f"pe_{i}{j}")
                    prod_mms(eps_, WTfin[:, 0], p3_sb[:, 3 * i + j])
                    nc.scalar.copy(flat(Wc_sb[:, 3 * i + j]), eps_[:])

    # ================= phase 4: conv =================
    with tc.tile_pool(name="xpool", bufs=6) as xpool, \
         tc.tile_pool(name="opool", bufs=3) as opool, \
         tc.tile_pool(name="psC", bufs=6, space="PSUM") as psC:
        for b in range(B_CORE):
            xp = []
            for tr in range(2):
                xpt = xpool.tile([P, PH, PH], F32R, tag="xp", name=f"xp_{b}_{tr}")
                nc.sync.dma_start(xpt[:, 1:65, 1:65], xs[b, tr * P:(tr + 1) * P, :, :])
                # circular halo: columns first (corners propagate via rows)
                nc.vector.tensor_copy(xpt[:, 1:65, 0:1], xpt[:, 1:65, 64:65])
                nc.vector.tensor_copy(xpt[:, 1:65, 65:66], xpt[:, 1:65, 1:2])
                nc.vector.tensor_copy(xpt[:, 0:1, :], xpt[:, 64:65, :])
                nc.vector.tensor_copy(xpt[:, 65:66, :], xpt[:, 1:2, :])
                xp.append(xpt)
            for mt in range(2):
                osb = opool.tile([P, NPIX], F32, tag="osb", name=f"osb_{b}_{mt}")
                for pb in range(8):
                    ops = psC.tile([P, 512], F32, tag="o")
                    idx = 0
                    for kh in range(3):
                        for kw in range(3):
                            for tr in range(2):
                                nc.tensor.matmul(
                                    ops[:],
                                    Wc_sb[:, 3 * kw + kh, tr, mt * P:(mt + 1) * P],
                                    xp[tr][:, pb * 8 + kh:pb * 8 + kh + 8,
                                           kw:kw + 64],
                                    start=(idx == 0), stop=(idx == 17))
                                idx += 1
                    nc.scalar.activation(
                        osb[:, pb * 512:(pb + 1) * 512], ops[:], ACTF.Identity,
                        bias=bias_sb[:, mt, :], scale=1.0)
                    nc.sync.dma_start(
                        out_ap[b, mt * P:(mt + 1) * P, pb * 8:(pb + 1) * 8, :],
                        osb[:, pb * 512:(pb + 1) * 512].rearrange(
                            "p (h w) -> p h w", w=H))


def build_program():
    from contextlib import ExitStack
    nc = bacc.Bacc("TRN2", target_bir_lowering=False, debug=False,
                   enable_asserts=False, num_devices=N_CORES)
    xs = nc.dram_tensor("xs", [B_CORE, C, H, H], F32R, kind="ExternalInput").ap()
    pm = nc.dram_tensor("pm", [NK, C, C], F32R, kind="ExternalInput").ap()
    u0 = nc.dram_tensor("u0", [NK, C, 1], F32R, kind="ExternalInput").ap()
    bias = nc.dram_tensor("bias", [C], F32, kind="ExternalInput").ap()
    out = nc.dram_tensor("out", [B_CORE, C, H, H], F32, kind="ExternalOutput").ap()
    with tile.TileContext(nc) as tc:
        with ExitStack() as ctx:
            build_body(tc, out, xs, pm, u0, bias, ctx)
    nc.compile()
    return nc


_cached_nc = None


def kernel(x, param_matrices, init_u, bias):
    global _cached_nc
    if _cached_nc is None:
        _cached_nc = build_program()
    nc = _cached_nc
    x = np.ascontiguousarray(np.asarray(x, dtype=np.float32))
    pm = np.ascontiguousarray(np.asarray(param_matrices, dtype=np.float32))
    u0 = np.ascontiguousarray(np.asarray(init_u, dtype=np.float32))
    b = np.ascontiguousarray(np.asarray(bias, dtype=np.float32))
    in_maps = [
        {"xs": x[i * B_CORE:(i + 1) * B_CORE], "pm": pm, "u0": u0, "bias": b}
        for i in range(N_CORES)
    ]
    res = run_bass_kernel_spmd(nc, in_maps, core_ids=list(range(N_CORES)))
    return np.concatenate([r["out"] for r in res.results], axis=0)


if __name__ == "__main__":
    import reference
    inputs = {k: np.asarray(v) for k, v in reference.setup_inputs().items()}
    out = kernel(**inputs)
    print(out.shape, out.dtype)


# revision 10
# speedup vs baseline: 1.1746x; 1.0582x over previous
"""BCOP (block-convolution orthogonal parameterization) forward on 8 TRN2 cores.

Math (validated vs reference in fp32 numpy):
  - power iteration via repeated squaring: G = A^T A; with v1 = A^T u0,
    d0 = v1.(G^18 v1), d1 = v1.(G^19 v1) reproduce the reference's
    normalized-power-iteration sigma: s = sqrt(d1/d0); G^18 v1 = G16@(G2@v1).
  - W0 = A/s; 20 Bjorck iters maintaining W and WT = W^T:
      G = W^T W  (lhsT=W,rhs=W);  M = 1.5 I - 0.5 G  (symmetric)
      W' = W M   (lhsT=WT,rhs=M); WT' = M WT (lhsT=M,rhs=WT)
  - downstream needs only WT = ortho^T:
      Z^T_i = WT[i+1] with rows>=128 zeroed -> PQ_i = matmul(lhsT=Z^T,rhs=Z^T)
      b1T[i1,j1] = (block_orth(PQ0,PQ1)[i1,j1])^T and b2 = block_orth(PQ2,PQ3),
      all products of symmetric matrices -> no transposes needed.
      p3[i,j] = sum b1[i1,j1] @ b2[i-i1,j-j1] = matmul(lhsT=b1T[..], rhs=b2[..])
      p_e[i,j] = H @ p3[i,j] = matmul(lhsT=WT[0], rhs=p3[i,j])
  - conv tap (kh,kw) uses stationary lhsT[ci,co] = p_e[kw,kh]; x circularly
    padded to 66x66 in SBUF; 18 accumulating matmuls (9 taps x 2 ci-tiles)
    per [128co x 512px] PSUM tile; bias added on the PSUM->SBUF evacuation.

PSUM discipline: every accumulation group owns a whole bank; a [128,512] tile
holds both 128-row output halves of a 256x256 product as ONE group (start=True
on the first matmul clears the whole zero-region; the other half's first
touch then writes fresh per-element).

Sharding: data-parallel over batch, 4 images per core; params + weight
construction replicated on all 8 cores; no collectives.
"""

import numpy as np

import concourse.bass as bass
import concourse.mybir as mybir
import concourse.tile as tile
from concourse import bacc
from concourse.bass_utils import run_bass_kernel_spmd
from concourse.masks import make_identity

P = 128
C = 256
NK = 5
N_CORES = 8
B_TOTAL = 32
B_CORE = B_TOTAL // N_CORES
H = 64
PH = 66
NPIX = H * H
BJORCK_ITERS = 20

F32 = mybir.dt.float32
F32R = mybir.dt.float32r
ALU = mybir.AluOpType
ACTF = mybir.ActivationFunctionType


def build_body(tc, out_ap, xs, pm, u0, bias_ap, ctx):
    nc = tc.nc

    persist = ctx.enter_context(tc.tile_pool(name="persist", bufs=1))
    small = ctx.enter_context(tc.tile_pool(name="small", bufs=3))

    U_sb = persist.tile([P, NK, 2, 1], F32R)
    for k in range(NK):
        for tr in range(2):
            nc.sync.dma_start(U_sb[:, k, tr, :], u0[k, tr * P:(tr + 1) * P, :])
    bias_sb = persist.tile([P, 2, 1], F32)
    for mt in range(2):
        nc.sync.dma_start(bias_sb[:, mt, :], bias_ap[mt * P:(mt + 1) * P].unsqueeze(1))

    # ---- constants (DVE/ACT; gpsimd only for the small 128x128 identity) ----
    ID1 = persist.tile([P, P], F32)
    make_identity(nc, ID1)
    I15 = persist.tile([P, 2, C], F32)
    I10 = persist.tile([P, 2, C], F32)
    nc.vector.memset(I15[:], 0.0)
    nc.vector.memset(I10[:], 0.0)
    for mt in range(2):
        nc.scalar.mul(I15[:, mt, mt * P:(mt + 1) * P], ID1[:], 1.5)
        nc.scalar.mul(I10[:, mt, mt * P:(mt + 1) * P], ID1[:], 1.0)
    I15f = I15.rearrange("p a b -> p (a b)")

    RB = persist.tile([P, NK], F32)       # broadcast 1/s per matrix
    Wc_sb = persist.tile([P, 9, 2, C], F32R)  # final conv lhsT tiles, slot (i,j)

    def flat(ap3):
        """[P, 2, C] -> [P, 512]"""
        return ap3.rearrange("p a b -> p (a b)")

    def prod_mms(out_ps, X3, Y3, n_tr=2):
        """One 256x256 product dst = X^T-style: out[mt] += X[tr][:, mt]^T @
        Y[tr]; X3/Y3 are [P, 2, C] APs; ONE accumulation group per bank."""
        first = True
        for mt in range(2):
            for tr in range(n_tr):
                last = (mt == 1 and tr == n_tr - 1)
                nc.tensor.matmul(out_ps[:, mt * C:(mt + 1) * C],
                                 X3[:, tr, mt * P:(mt + 1) * P], Y3[:, tr, :],
                                 start=first, stop=last)
                first = False

    with tc.tile_pool(name="build", bufs=1) as build, \
         tc.tile_pool(name="wstate", bufs=2) as wpool, \
         tc.tile_pool(name="mpool", bufs=4) as mpool, \
         tc.tile_pool(name="vpool", bufs=8) as vpool:

        Wcur = wpool.tile([P, NK, 2, C], F32R, tag="W")
        WTcur = wpool.tile([P, NK, 2, C], F32R, tag="WT")
        G_sb = build.tile([P, NK, 2, C], F32R, tag="big")  # slot shared with p3
        G2_sb = build.tile([P, NK, 2, C], F32R)
        G16_sb = build.tile([P, NK, 2, C], F32R)

        for k in range(NK):
            for tr in range(2):
                nc.sync.dma_start(Wcur[:, k, tr, :], pm[k, tr * P:(tr + 1) * P, :])

        # ============ phase 1: sigma via repeated squaring ============
        with tc.tile_pool(name="ps1", bufs=2, space="PSUM") as ps1:
            for k in range(NK):
                # G = A^T A ; then G2 -> G4 -> G8 -> G16 by squaring
                gps = ps1.tile([P, 2 * C], F32, tag="sq")
                prod_mms(gps, Wcur[:, k], Wcur[:, k])
                nc.scalar.copy(flat(G_sb[:, k]), gps[:])

                prev = G_sb[:, k]
                for pw in (2, 4, 8, 16):
                    sq = ps1.tile([P, 2 * C], F32, tag="sq", name=f"sq{pw}_{k}")
                    prod_mms(sq, prev, prev)
                    if pw == 2:
                        dst = G2_sb[:, k]
                    elif pw == 16:
                        dst = G16_sb[:, k]
                    else:
                        dst = build.tile([P, 2, C], F32R, tag="gtmp",
                                         name=f"g{pw}_{k}", bufs=2)
                    if pw in (4, 16):
                        nc.vector.tensor_copy(flat(dst), sq[:])
                    else:
                        nc.scalar.copy(flat(dst), sq[:])
                    prev = dst

                # v1 = A^T u0 (plain fp32; tiny)
                def matvec(G3, vin, nm):
                    vout = vpool.tile([P, 2], F32R, tag="v", name=f"v_{nm}")
                    for mt in range(2):
                        vps = ps1.tile([P, 1], F32, tag="vps", bufs=4,
                                       name=f"vp_{nm}_{mt}")
                        for tr in range(2):
                            nc.tensor.matmul(
                                vps[:], G3[:, tr, mt * P:(mt + 1) * P].bitcast(F32),
                                vin[:, tr:tr + 1].bitcast(F32),
                                start=(tr == 0), stop=(tr == 1))
                        nc.scalar.copy(vout[:, mt:mt + 1], vps[:])
                    return vout

                v1 = matvec(Wcur[:, k], U_sb[:, k], f"v1_{k}")
                m1 = matvec(G2_sb[:, k], v1, f"m1_{k}")
                m2 = matvec(G16_sb[:, k], m1, f"m2_{k}")
                m3 = matvec(G_sb[:, k], m2, f"m3_{k}")

                # d0 = v1.m2 ; d1 = v1.m3 ; r = sqrt(d0/d1) = 1/s
                def dot(va, vb, nm):
                    dps = ps1.tile([1, 1], F32, tag="vps", bufs=4, name=f"d_{nm}")
                    for tr in range(2):
                        nc.tensor.matmul(dps[:], va[:, tr:tr + 1].bitcast(F32),
                                         vb[:, tr:tr + 1].bitcast(F32),
                                         start=(tr == 0), stop=(tr == 1))
                    return dps

                dps0 = dot(v1, m2, f"0_{k}")
                dps1 = dot(v1, m3, f"1_{k}")
                dsb = small.tile([1, 3], F32, tag="dsb")
                nc.vector.tensor_copy(dsb[:, 0:1], dps0[:])
                nc.vector.reciprocal(dsb[:, 1:2], dps1[:])
                nc.vector.tensor_mul(dsb[:, 2:3], dsb[:, 0:1], dsb[:, 1:2])
                rsb = small.tile([1, 1], F32, tag="rsb")
                nc.scalar.sqrt(rsb[:], dsb[:, 2:3])
                nc.gpsimd.partition_broadcast(RB[:, k:k + 1], rsb[:])
                # W0 = A * r (in place), then WT0 = W0^T via PE transpose
                nc.vector.tensor_scalar_mul(Wcur[:, k], Wcur[:, k], RB[:, k:k + 1])
                for tr in range(2):
                    for mt in range(2):
                        tps = ps1.tile([P, P], F32, tag="tp")
                        nc.tensor.transpose(
                            tps[:], Wcur[:, k, tr, mt * P:(mt + 1) * P].bitcast(F32),
                            ID1[:])
                        nc.scalar.copy(WTcur[:, k, mt, tr * P:(tr + 1) * P], tps[:])

        # ================= phase 2: Bjorck =================
        with tc.tile_pool(name="ps2", bufs=2, space="PSUM") as ps2:
            for it in range(BJORCK_ITERS):
                last = it == BJORCK_ITERS - 1
                Wnxt = None if last else wpool.tile([P, NK, 2, C], F32R, tag="W",
                                                    name=f"W_{it}")
                WTnxt = wpool.tile([P, NK, 2, C], F32R, tag="WT", name=f"WT_{it}")
                for k in range(NK):
                    gps = ps2.tile([P, 2 * C], F32, tag="g", bufs=3)
                    prod_mms(gps, Wcur[:, k], Wcur[:, k])
                    m_sb = mpool.tile([P, 2 * C], F32R, tag="m", name=f"m_{it}_{k}")
                    nc.vector.scalar_tensor_tensor(
                        m_sb[:], gps[:], -0.5, I15f, op0=ALU.mult, op1=ALU.add)
                    m3 = m_sb.rearrange("p (a b) -> p a b", b=C)
                    if not last:
                        wps = ps2.tile([P, 2 * C], F32, tag="w")
                        prod_mms(wps, WTcur[:, k], m3)
                        nc.scalar.copy(flat(Wnxt[:, k]), wps[:])
                    wtps = ps2.tile([P, 2 * C], F32, tag="wt", bufs=3)
                    prod_mms(wtps, m3, WTcur[:, k])
                    nc.scalar.copy(flat(WTnxt[:, k]), wtps[:])
                if Wnxt is not None:
                    Wcur = Wnxt
                WTcur = WTnxt
        WTfin = WTcur

        # ================= phase 3: weight assembly =================
        PQ_sb = build.tile([P, 4, 2, C], F32R)
        IP_sb = build.tile([P, 4, 2, C], F32R)
        b1T_sb = build.tile([P, 2, 2, 2, C], F32R)
        b2_sb = build.tile([P, 2, 2, 2, C], F32R)
        p3_sb = build.tile([P, 9, 2, C], F32R, tag="big")  # reuses G_sb's slot

        with tc.tile_pool(name="ps3", bufs=4, space="PSUM") as ps3:
            # PQ_i = Z Z^T (contract only first 128 rows of WT[i+1]) ; IP = I-PQ
            for i in range(4):
                qps = ps3.tile([P, 2 * C], F32, tag="as", name=f"q_{i}")
                prod_mms(qps, WTfin[:, i + 1], WTfin[:, i + 1], n_tr=1)
                nc.scalar.copy(flat(PQ_sb[:, i]), qps[:])
                nc.vector.tensor_sub(flat(IP_sb[:, i]), flat(I10), flat(PQ_sb[:, i]))

            def symprod(dst3, X3, Y3, nm):
                bps = ps3.tile([P, 2 * C], F32, tag="as", name=f"bp_{nm}")
                prod_mms(bps, X3, Y3)
                nc.scalar.copy(flat(dst3), bps[:])

            # b1T[i1,j1] = (j1? IP1:PQ1) @ (i1? IP0:PQ0)
            for i1 in range(2):
                for j1 in range(2):
                    symprod(b1T_sb[:, i1, j1],
                            IP_sb[:, 1] if j1 else PQ_sb[:, 1],
                            IP_sb[:, 0] if i1 else PQ_sb[:, 0], f"b1T{i1}{j1}")
            # b2[i2,j2] = (i2? IP2:PQ2) @ (j2? IP3:PQ3)
            for i2 in range(2):
                for j2 in range(2):
                    symprod(b2_sb[:, i2, j2],
                            IP_sb[:, 2] if i2 else PQ_sb[:, 2],
                            IP_sb[:, 3] if j2 else PQ_sb[:, 3], f"b2{i2}{j2}")

            # p3[i,j] = sum_{i1,j1} b1[i1,j1] @ b2[i-i1,j-j1]
            for i in range(3):
                for j in range(3):
                    terms = [(i1, j1) for i1 in range(2) for j1 in range(2)
                             if 0 <= i - i1 < 2 and 0 <= j - j1 < 2]
                    pps = ps3.tile([P, 2 * C], F32, tag="as", name=f"p3_{i}{j}")
                    nmm = len(terms) * 4
                    idx = 0
                    for mt in range(2):
                        for (i1, j1) in terms:
                            for tr in range(2):
                                idx += 1
                                nc.tensor.matmul(
                                    pps[:, mt * C:(mt + 1) * C],
                                    b1T_sb[:, i1, j1, tr, mt * P:(mt + 1) * P],
                                    b2_sb[:, i - i1, j - j1, tr, :],
                                    start=(idx == 1), stop=(idx == nmm))
                    nc.vector.tensor_copy(flat(p3_sb[:, 3 * i + j]), pps[:])

            # p_e[i,j] = H @ p3[i,j]  (lhsT = H^T = WTfin[0])
            for i in range(3):
                for j in range(3):
                    eps_ = ps3.tile([P, 2 * C], F32, tag="as", name=f"pe_{i}{j}")
                    prod_mms(eps_, WTfin[:, 0], p3_sb[:, 3 * i + j])
                    nc.scalar.copy(flat(Wc_sb[:, 3 * i + j]), eps_[:])

    # ================= phase 4: conv =================
    with tc.tile_pool(name="xpool", bufs=6) as xpool, \
         tc.tile_pool(name="opool", bufs=3) as opool, \
         tc.tile_pool(name="psC", bufs=8, space="PSUM") as psC:
        for b in range(B_CORE):
            xp = []
            for tr in range(2):
                xpt = xpool.tile([P, PH, PH], F32R, tag="xp", name=f"xp_{b}_{tr}")
                nc.sync.dma_start(xpt[:, 1:65, 1:65], xs[b, tr * P:(tr + 1) * P, :, :])
                # circular halo: columns first (corners propagate via rows)
                nc.vector.tensor_copy(xpt[:, 1:65, 0:1], xpt[:, 1:65, 64:65])
                nc.vector.tensor_copy(xpt[:, 1:65, 65:66], xpt[:, 1:65, 1:2])
                nc.vector.tensor_copy(xpt[:, 0:1, :], xpt[:, 64:65, :])
                nc.vector.tensor_copy(xpt[:, 65:66, :], xpt[:, 1:2, :])
                xp.append(xpt)
            for mt in range(2):
                osb = opool.tile([P, NPIX], F32, tag="osb", name=f"osb_{b}_{mt}")
                for pb in range(8):
                    ops = psC.tile([P, 512], F32, tag="o")
                    idx = 0
                    for kh in range(3):
                        for kw in range(3):
                            for tr in range(2):
                                nc.tensor.matmul(
                                    ops[:],
                                    Wc_sb[:, 3 * kw + kh, tr, mt * P:(mt + 1) * P],
                                    xp[tr][:, pb * 8 + kh:pb * 8 + kh + 8,
                                           kw:kw + 64],
                                    start=(idx == 0), stop=(idx == 17))
                                idx += 1
                    nc.scalar.activation(
                        osb[:, pb * 512:(pb + 1) * 512], ops[:], ACTF.Identity,
                        bias=bias_sb[:, mt, :], scale=1.0)
                    nc.sync.dma_start(
                        out_ap[b, mt * P:(mt + 1) * P, pb * 8:(pb + 1) * 8, :],
                        osb[:, pb * 512:(pb + 1) * 512].rearrange(
                            "p (h w) -> p h w", w=H))


def build_program():
    from contextlib import ExitStack
    nc = bacc.Bacc("TRN2", target_bir_lowering=False, debug=False,
                   enable_asserts=False, num_devices=N_CORES)
    xs = nc.dram_tensor("xs", [B_CORE, C, H, H], F32R, kind="ExternalInput").ap()
    pm = nc.dram_tensor("pm", [NK, C, C], F32R, kind="ExternalInput").ap()
    u0 = nc.dram_tensor("u0", [NK, C, 1], F32R, kind="ExternalInput").ap()
    bias = nc.dram_tensor("bias", [C], F32, kind="ExternalInput").ap()
    out = nc.dram_tensor("out", [B_CORE, C, H, H], F32, kind="ExternalOutput").ap()
    with tile.TileContext(nc) as tc:
        with ExitStack() as ctx:
            build_body(tc, out, xs, pm, u0, bias, ctx)
    nc.compile()
    return nc


_cached_nc = None


def kernel(x, param_matrices, init_u, bias):
    global _cached_nc
    if _cached_nc is None:
        _cached_nc = build_program()
    nc = _cached_nc
    x = np.ascontiguousarray(np.asarray(x, dtype=np.float32))
    pm = np.ascontiguousarray(np.asarray(param_matrices, dtype=np.float32))
    u0 = np.ascontiguousarray(np.asarray(init_u, dtype=np.float32))
    b = np.ascontiguousarray(np.asarray(bias, dtype=np.float32))
    in_maps = [
        {"xs": x[i * B_CORE:(i + 1) * B_CORE], "pm": pm, "u0": u0, "bias": b}
        for i in range(N_CORES)
    ]
    res = run_bass_kernel_spmd(nc, in_maps, core_ids=list(range(N_CORES)))
    return np.concatenate([r["out"] for r in res.results], axis=0)


if __name__ == "__main__":
    import reference
    inputs = {k: np.asarray(v) for k, v in reference.setup_inputs().items()}
    out = kernel(**inputs)
    print(out.shape, out.dtype)


# revision 12
# speedup vs baseline: 1.3051x; 1.1111x over previous
"""BCOP (block-convolution orthogonal parameterization) forward on 8 TRN2 cores.

Math (validated vs reference in fp32 numpy):
  - power iteration via repeated squaring: G = A^T A; with v1 = A^T u0,
    d0 = v1.(G^18 v1), d1 = v1.(G^19 v1) reproduce the reference's
    normalized-power-iteration sigma: s = sqrt(d1/d0); G^18 v1 = G16@(G2@v1).
  - W0 = A/s; 20 Bjorck iters maintaining W and WT = W^T:
      G = W^T W  (lhsT=W,rhs=W);  M = 1.5 I - 0.5 G  (symmetric)
      W' = W M   (lhsT=WT,rhs=M); WT' = M WT (lhsT=M,rhs=WT)
  - downstream needs only WT = ortho^T:
      Z^T_i = WT[i+1] with rows>=128 zeroed -> PQ_i = matmul(lhsT=Z^T,rhs=Z^T)
      b1T[i1,j1] = (block_orth(PQ0,PQ1)[i1,j1])^T and b2 = block_orth(PQ2,PQ3),
      all products of symmetric matrices -> no transposes needed.
      p3[i,j] = sum b1[i1,j1] @ b2[i-i1,j-j1] = matmul(lhsT=b1T[..], rhs=b2[..])
      p_e[i,j] = H @ p3[i,j] = matmul(lhsT=WT[0], rhs=p3[i,j])
  - conv tap (kh,kw) uses stationary lhsT[ci,co] = p_e[kw,kh]; x circularly
    padded to 66x66 in SBUF; 18 accumulating matmuls (9 taps x 2 ci-tiles)
    per [128co x 512px] PSUM tile; bias added on the PSUM->SBUF evacuation.

Parallelization: the per-matrix weight construction (power iteration + Bjorck)
is INDEPENDENT across the 5 parameter matrices, so it is sharded across cores
via the inputs (core i receives matrix i mod 5 only), then an 8-rank AllGather
shares the 5 WT results; assembly + conv then run replicated. The conv is
data-parallel over batch (4 images per core).

PSUM discipline: every accumulation group owns a whole bank; a [128,512] tile
holds both 128-row output halves of a 256x256 product as ONE group.
"""

import numpy as np

import concourse.bass as bass
import concourse.mybir as mybir
import concourse.tile as tile
from concourse import bacc
from concourse.bass_utils import run_bass_kernel_spmd

P = 128
C = 256
NK = 5
N_CORES = 8
B_TOTAL = 32
B_CORE = B_TOTAL // N_CORES
H = 64
PH = 66
NPIX = H * H
BJORCK_ITERS = 20

F32 = mybir.dt.float32
F32R = mybir.dt.float32r
ALU = mybir.AluOpType
ACTF = mybir.ActivationFunctionType


def build_body(tc, out_ap, xs, pmk, u0k, bias_ap, ctx):
    nc = tc.nc
    from concourse.masks import make_identity

    persist = ctx.enter_context(tc.tile_pool(name="persist", bufs=1))
    small = ctx.enter_context(tc.tile_pool(name="small", bufs=3))

    U_sb = persist.tile([P, 2, 1], F32R)
    for tr in range(2):
        nc.sync.dma_start(U_sb[:, tr, :], u0k[tr * P:(tr + 1) * P, :])
    bias_sb = persist.tile([P, 2, 1], F32)
    for mt in range(2):
        nc.sync.dma_start(bias_sb[:, mt, :], bias_ap[mt * P:(mt + 1) * P].unsqueeze(1))

    # ---- constants ----
    ID1 = persist.tile([P, P], F32)
    make_identity(nc, ID1)
    I15 = persist.tile([P, 2, C], F32)
    I10 = persist.tile([P, 2, C], F32)
    nc.vector.memset(I15[:], 0.0)
    nc.vector.memset(I10[:], 0.0)
    for mt in range(2):
        nc.scalar.mul(I15[:, mt, mt * P:(mt + 1) * P], ID1[:], 1.5)
        nc.scalar.mul(I10[:, mt, mt * P:(mt + 1) * P], ID1[:], 1.0)
    I15f = I15.rearrange("p a b -> p (a b)")

    RB = persist.tile([P, 1], F32)            # broadcast 1/s (own matrix)
    Wc_sb = persist.tile([P, 9, 2, C], F32R)  # final conv lhsT tiles, slot (i,j)
    WTfin = persist.tile([P, NK, 2, C], F32R)  # gathered ortho^T, all matrices

    def flat(ap3):
        return ap3.rearrange("p a b -> p (a b)")

    def prod_mms(out_ps, X3, Y3, n_tr=2):
        """dst[mt] += X[tr][:, mt]^T @ Y[tr]; ONE accumulation group per bank."""
        first = True
        for mt in range(2):
            for tr in range(n_tr):
                last = (mt == 1 and tr == n_tr - 1)
                nc.tensor.matmul(out_ps[:, mt * C:(mt + 1) * C],
                                 X3[:, tr, mt * P:(mt + 1) * P], Y3[:, tr, :],
                                 start=first, stop=last)
                first = False

    with tc.tile_pool(name="build", bufs=1) as build, \
         tc.tile_pool(name="wstate", bufs=2) as wpool, \
         tc.tile_pool(name="mpool", bufs=4) as mpool, \
         tc.tile_pool(name="vpool", bufs=8) as vpool, \
         tc.tile_pool(name="ccdram", bufs=1, space="DRAM") as ccdram:

        Wcur = wpool.tile([P, 2, C], F32R, tag="W")
        WTcur = wpool.tile([P, 2, C], F32R, tag="WT")
        G_sb = build.tile([P, 2, C], F32R)
        G2_sb = build.tile([P, 2, C], F32R)
        G16_sb = build.tile([P, 2, C], F32R)

        for tr in range(2):
            nc.sync.dma_start(Wcur[:, tr, :], pmk[tr * P:(tr + 1) * P, :])

        # ============ phase 1: sigma via repeated squaring (own matrix) ======
        with tc.tile_pool(name="ps1", bufs=2, space="PSUM") as ps1:
            gps = ps1.tile([P, 2 * C], F32, tag="sq")
            prod_mms(gps, Wcur, Wcur)
            nc.scalar.copy(flat(G_sb), gps[:])

            prev = G_sb
            for pw in (2, 4, 8, 16):
                sq = ps1.tile([P, 2 * C], F32, tag="sq", name=f"sq{pw}")
                prod_mms(sq, prev, prev)
                if pw == 2:
                    dst = G2_sb
                elif pw == 16:
                    dst = G16_sb
                else:
                    dst = build.tile([P, 2, C], F32R, tag="gtmp",
                                     name=f"g{pw}", bufs=2)
                if pw in (4, 16):
                    nc.vector.tensor_copy(flat(dst), sq[:])
                else:
                    nc.scalar.copy(flat(dst), sq[:])
                prev = dst

            def matvec(G3, vin, nm):
                vout = vpool.tile([P, 2], F32R, tag="v", name=f"v_{nm}")
                for mt in range(2):
                    vps = ps1.tile([P, 1], F32, tag="vps", bufs=4,
                                   name=f"vp_{nm}_{mt}")
                    for tr in range(2):
                        nc.tensor.matmul(
                            vps[:], G3[:, tr, mt * P:(mt + 1) * P].bitcast(F32),
                            vin[:, tr:tr + 1].bitcast(F32),
                            start=(tr == 0), stop=(tr == 1))
                    nc.scalar.copy(vout[:, mt:mt + 1], vps[:])
                return vout

            v1 = matvec(Wcur, U_sb, "v1")
            m1 = matvec(G2_sb, v1, "m1")
            m2 = matvec(G16_sb, m1, "m2")
            m3 = matvec(G_sb, m2, "m3")

            def dot(va, vb, nm):
                dps = ps1.tile([1, 1], F32, tag="vps", bufs=4, name=f"d_{nm}")
                for tr in range(2):
                    nc.tensor.matmul(dps[:], va[:, tr:tr + 1].bitcast(F32),
                                     vb[:, tr:tr + 1].bitcast(F32),
                                     start=(tr == 0), stop=(tr == 1))
                return dps

            dps0 = dot(v1, m2, "0")
            dps1 = dot(v1, m3, "1")
            dsb = small.tile([1, 3], F32, tag="dsb")
            nc.vector.tensor_copy(dsb[:, 0:1], dps0[:])
            nc.vector.reciprocal(dsb[:, 1:2], dps1[:])
            nc.vector.tensor_mul(dsb[:, 2:3], dsb[:, 0:1], dsb[:, 1:2])
            rsb = small.tile([1, 1], F32, tag="rsb")
            nc.scalar.sqrt(rsb[:], dsb[:, 2:3])
            nc.gpsimd.partition_broadcast(RB[:, 0:1], rsb[:])
            # W0 = A * r (in place), then WT0 = W0^T via PE transpose
            nc.vector.tensor_scalar_mul(Wcur[:], Wcur[:], RB[:, 0:1])
            for tr in range(2):
                for mt in range(2):
                    tps = ps1.tile([P, P], F32, tag="tp")
                    nc.tensor.transpose(
                        tps[:], Wcur[:, tr, mt * P:(mt + 1) * P].bitcast(F32),
                        ID1[:])
                    nc.scalar.copy(WTcur[:, mt, tr * P:(tr + 1) * P], tps[:])

        # ================= phase 2: Bjorck (own matrix) =================
        with tc.tile_pool(name="ps2", bufs=2, space="PSUM") as ps2:
            for it in range(BJORCK_ITERS):
                last = it == BJORCK_ITERS - 1
                Wnxt = None if last else wpool.tile([P, 2, C], F32R, tag="W",
                                                    name=f"W_{it}")
                WTnxt = wpool.tile([P, 2, C], F32R, tag="WT", name=f"WT_{it}")
                gps = ps2.tile([P, 2 * C], F32, tag="g", bufs=2)
                prod_mms(gps, Wcur, Wcur)
                m_sb = mpool.tile([P, 2 * C], F32R, tag="m", name=f"m_{it}")
                nc.vector.scalar_tensor_tensor(
                    m_sb[:], gps[:], -0.5, I15f, op0=ALU.mult, op1=ALU.add)
                m3 = m_sb.rearrange("p (a b) -> p a b", b=C)
                if not last:
                    wps = ps2.tile([P, 2 * C], F32, tag="w", bufs=2)
                    prod_mms(wps, WTcur, m3)
                    nc.scalar.copy(flat(Wnxt), wps[:])
                wtps = ps2.tile([P, 2 * C], F32, tag="wt", bufs=2)
                prod_mms(wtps, m3, WTcur)
                nc.vector.tensor_copy(flat(WTnxt), wtps[:])
                if Wnxt is not None:
                    Wcur = Wnxt
                WTcur = WTnxt

        # ============ AllGather the 5 WT matrices across cores ============
        cc_in = ccdram.tile([1, P * 2 * C], F32R)
        cc_out = ccdram.tile([N_CORES, P * 2 * C], F32R, addr_space="Shared")
        nc.sync.dma_start(cc_in[0].rearrange("(p n) -> p n", p=P), flat(WTcur))
        nc.gpsimd.collective_compute(
            "AllGather", ALU.bypass, ins=[cc_in.opt()], outs=[cc_out.opt()],
            replica_groups=[list(range(N_CORES))])
        for k in range(NK):
            nc.sync.dma_start(flat(WTfin[:, k]),
                              cc_out[k].rearrange("(p n) -> p n", p=P))

        # ================= phase 3: weight assembly =================
        PQ_sb = build.tile([P, 4, 2, C], F32R)
        IP_sb = build.tile([P, 4, 2, C], F32R)
        b1T_sb = build.tile([P, 2, 2, 2, C], F32R)
        b2_sb = build.tile([P, 2, 2, 2, C], F32R)
        p3_sb = build.tile([P, 9, 2, C], F32R)

        with tc.tile_pool(name="ps3", bufs=4, space="PSUM") as ps3:
            for i in range(4):
                qps = ps3.tile([P, 2 * C], F32, tag="as", name=f"q_{i}")
                prod_mms(qps, WTfin[:, i + 1], WTfin[:, i + 1], n_tr=1)
                nc.scalar.copy(flat(PQ_sb[:, i]), qps[:])
                nc.vector.tensor_sub(flat(IP_sb[:, i]), flat(I10), flat(PQ_sb[:, i]))

            def symprod(dst3, X3, Y3, nm):
                bps = ps3.tile([P, 2 * C], F32, tag="as", name=f"bp_{nm}")
                prod_mms(bps, X3, Y3)
                nc.scalar.copy(flat(dst3), bps[:])

            for i1 in range(2):
                for j1 in range(2):
                    symprod(b1T_sb[:, i1, j1],
                            IP_sb[:, 1] if j1 else PQ_sb[:, 1],
                            IP_sb[:, 0] if i1 else PQ_sb[:, 0], f"b1T{i1}{j1}")
            for i2 in range(2):
                for j2 in range(2):
                    symprod(b2_sb[:, i2, j2],
                            IP_sb[:, 2] if i2 else PQ_sb[:, 2],
                            IP_sb[:, 3] if j2 else PQ_sb[:, 3], f"b2{i2}{j2}")

            for i in range(3):
                for j in range(3):
                    terms = [(i1, j1) for i1 in range(2) for j1 in range(2)
                             if 0 <= i - i1 < 2 and 0 <= j - j1 < 2]
                    pps = ps3.tile([P, 2 * C], F32, tag="as", name=f"p3_{i}{j}")
                    nmm = len(terms) * 4
                    idx = 0
                    for mt in range(2):
                        for (i1, j1) in terms:
                            for tr in range(2):
                                idx += 1
                                nc.tensor.matmul(
                                    pps[:, mt * C:(mt + 1) * C],
                                    b1T_sb[:, i1, j1, tr, mt * P:(mt + 1) * P],
                                    b2_sb[:, i - i1, j - j1, tr, :],
                                    start=(idx == 1), stop=(idx == nmm))
                    nc.vector.tensor_copy(flat(p3_sb[:, 3 * i + j]), pps[:])

            for i in range(3):
                for j in range(3):
                    eps_ = ps3.tile([P, 2 * C], F32, tag="as", name=f"pe_{i}{j}")
                    prod_mms(eps_, WTfin[:, 0], p3_sb[:, 3 * i + j])
                    nc.scalar.copy(flat(Wc_sb[:, 3 * i + j]), eps_[:])

    # ================= phase 4: conv =================
    with tc.tile_pool(name="xpool", bufs=6) as xpool, \
         tc.tile_pool(name="opool", bufs=3) as opool, \
         tc.tile_pool(name="psC", bufs=8, space="PSUM") as psC:
        for b in range(B_CORE):
            xp = []
            for tr in range(2):
                xpt = xpool.tile([P, PH, PH], F32R, tag="xp", name=f"xp_{b}_{tr}")
                nc.sync.dma_start(xpt[:, 1:65, 1:65], xs[b, tr * P:(tr + 1) * P, :, :])
                nc.vector.tensor_copy(xpt[:, 1:65, 0:1], xpt[:, 1:65, 64:65])
                nc.vector.tensor_copy(xpt[:, 1:65, 65:66], xpt[:, 1:65, 1:2])
                nc.vector.tensor_copy(xpt[:, 0:1, :], xpt[:, 64:65, :])
                nc.vector.tensor_copy(xpt[:, 65:66, :], xpt[:, 1:2, :])
                xp.append(xpt)
            for mt in range(2):
                osb = opool.tile([P, NPIX], F32, tag="osb", name=f"osb_{b}_{mt}")
                for pb in range(8):
                    ops = psC.tile([P, 512], F32, tag="o")
                    idx = 0
                    for kh in range(3):
                        for kw in range(3):
                            for tr in range(2):
                                nc.tensor.matmul(
                                    ops[:],
                                    Wc_sb[:, 3 * kw + kh, tr, mt * P:(mt + 1) * P],
                                    xp[tr][:, pb * 8 + kh:pb * 8 + kh + 8,
                                           kw:kw + 64],
                                    start=(idx == 0), stop=(idx == 17))
                                idx += 1
                    nc.scalar.activation(
                        osb[:, pb * 512:(pb + 1) * 512], ops[:], ACTF.Identity,
                        bias=bias_sb[:, mt, :], scale=1.0)
                    nc.sync.dma_start(
                        out_ap[b, mt * P:(mt + 1) * P, pb * 8:(pb + 1) * 8, :],
                        osb[:, pb * 512:(pb + 1) * 512].rearrange(
                            "p (h w) -> p h w", w=H))


def build_program():
    from contextlib import ExitStack
    nc = bacc.Bacc("TRN2", target_bir_lowering=False, debug=False,
                   enable_asserts=False, num_devices=N_CORES)
    xs = nc.dram_tensor("xs", [B_CORE, C, H, H], F32R, kind="ExternalInput").ap()
    pmk = nc.dram_tensor("pmk", [C, C], F32R, kind="ExternalInput").ap()
    u0k = nc.dram_tensor("u0k", [C, 1], F32R, kind="ExternalInput").ap()
    bias = nc.dram_tensor("bias", [C], F32, kind="ExternalInput").ap()
    out = nc.dram_tensor("out", [B_CORE, C, H, H], F32, kind="ExternalOutput").ap()
    with tile.TileContext(nc) as tc:
        with ExitStack() as ctx:
            build_body(tc, out, xs, pmk, u0k, bias, ctx)
    nc.compile()
    return nc


_cached_nc = None


def make_in_maps(x, pm, u0, b):
    in_maps = []
    for i in range(N_CORES):
        k = i if i < NK else i - NK
        in_maps.append({
            "xs": np.ascontiguousarray(x[i * B_CORE:(i + 1) * B_CORE]),
            "pmk": np.ascontiguousarray(pm[k]),
            "u0k": np.ascontiguousarray(u0[k]),
            "bias": np.ascontiguousarray(b),
        })
    return in_maps


def kernel(x, param_matrices, init_u, bias):
    global _cached_nc
    if _cached_nc is None:
        _cached_nc = build_program()
    nc = _cached_nc
    x = np.ascontiguousarray(np.asarray(x, dtype=np.float32))
    pm = np.ascontiguousarray(np.asarray(param_matrices, dtype=np.float32))
    u0 = np.ascontiguousarray(np.asarray(init_u, dtype=np.float32))
    b = np.ascontiguousarray(np.asarray(bias, dtype=np.float32))
    in_maps = make_in_maps(x, pm, u0, b)
    res = run_bass_kernel_spmd(nc, in_maps, core_ids=list(range(N_CORES)))
    return np.concatenate([r["out"] for r in res.results], axis=0)


if __name__ == "__main__":
    import reference
    inputs = {k: np.asarray(v) for k, v in reference.setup_inputs().items()}
    out = kernel(**inputs)
    print(out.shape, out.dtype)


# revision 13
# speedup vs baseline: 1.3383x; 1.0254x over previous
"""BCOP (block-convolution orthogonal parameterization) forward on 8 TRN2 cores.

Math (validated vs reference in fp32 numpy):
  - power iteration via repeated squaring: G = A^T A; with v1 = A^T u0,
    d0 = v1.(G^18 v1), d1 = v1.(G^19 v1) reproduce the reference's
    normalized-power-iteration sigma: s = sqrt(d1/d0); G^18 v1 = G16@(G2@v1).
  - W0 = A/s; 20 Bjorck iters maintaining W and WT = W^T:
      G = W^T W  (lhsT=W,rhs=W);  M = 1.5 I - 0.5 G  (symmetric)
      W' = W M   (lhsT=WT,rhs=M); WT' = M WT (lhsT=M,rhs=WT)
  - downstream needs only WT = ortho^T:
      Z^T_i = WT[i+1] with rows>=128 zeroed -> PQ_i = matmul(lhsT=Z^T,rhs=Z^T)
      b1T[i1,j1] = (block_orth(PQ0,PQ1)[i1,j1])^T and b2 = block_orth(PQ2,PQ3),
      all products of symmetric matrices -> no transposes needed.
      p3[i,j] = sum b1[i1,j1] @ b2[i-i1,j-j1] = matmul(lhsT=b1T[..], rhs=b2[..])
      p_e[i,j] = H @ p3[i,j] = matmul(lhsT=WT[0], rhs=p3[i,j])
  - conv tap (kh,kw) uses stationary lhsT[ci,co] = p_e[kw,kh]; x circularly
    padded to 66x66 in SBUF; 18 accumulating matmuls (9 taps x 2 ci-tiles)
    per [128co x 512px] PSUM tile; bias added on the PSUM->SBUF evacuation.

Parallelization: the per-matrix weight construction (power iteration + Bjorck)
is INDEPENDENT across the 5 parameter matrices, so it is sharded across cores
via the inputs (core i receives matrix i mod 5 only), then an 8-rank AllGather
shares the 5 WT results; assembly + conv then run replicated. The conv is
data-parallel over batch (4 images per core).

PSUM discipline: every accumulation group owns a whole bank; a [128,512] tile
holds both 128-row output halves of a 256x256 product as ONE group.
"""

import numpy as np

import concourse.bass as bass
import concourse.mybir as mybir
import concourse.tile as tile
from concourse import bacc
from concourse.bass_utils import run_bass_kernel_spmd

P = 128
C = 256
NK = 5
N_CORES = 8
B_TOTAL = 32
B_CORE = B_TOTAL // N_CORES
H = 64
PH = 66
NPIX = H * H
BJORCK_ITERS = 20

F32 = mybir.dt.float32
F32R = mybir.dt.float32r
ALU = mybir.AluOpType
ACTF = mybir.ActivationFunctionType


def build_body(tc, out_ap, xs, pmk, u0k, bias_ap, ctx):
    nc = tc.nc
    from concourse.masks import make_identity

    persist = ctx.enter_context(tc.tile_pool(name="persist", bufs=1))
    small = ctx.enter_context(tc.tile_pool(name="small", bufs=3))

    U_sb = persist.tile([P, 2, 1], F32R)
    for tr in range(2):
        nc.sync.dma_start(U_sb[:, tr, :], u0k[tr * P:(tr + 1) * P, :])
    bias_sb = persist.tile([P, 2, 1], F32)
    for mt in range(2):
        nc.sync.dma_start(bias_sb[:, mt, :], bias_ap[mt * P:(mt + 1) * P].unsqueeze(1))

    # ---- constants ----
    ID1 = persist.tile([P, P], F32)
    make_identity(nc, ID1)
    I15 = persist.tile([P, 2, C], F32)
    I10 = persist.tile([P, 2, C], F32)
    nc.vector.memset(I15[:], 0.0)
    nc.vector.memset(I10[:], 0.0)
    for mt in range(2):
        nc.scalar.mul(I15[:, mt, mt * P:(mt + 1) * P], ID1[:], 1.5)
        nc.scalar.mul(I10[:, mt, mt * P:(mt + 1) * P], ID1[:], 1.0)
    I15f = I15.rearrange("p a b -> p (a b)")

    RB = persist.tile([P, 1], F32)            # broadcast 1/s (own matrix)
    Wc_sb = persist.tile([P, 9, 2, C], F32R)  # final conv lhsT tiles, slot (i,j)
    WTfin = persist.tile([P, NK, 2, C], F32R)  # gathered ortho^T, all matrices

    def flat(ap3):
        return ap3.rearrange("p a b -> p (a b)")

    def prod_mms(out_ps, X3, Y3, n_tr=2):
        """dst[mt] += X[tr][:, mt]^T @ Y[tr]; ONE accumulation group per bank."""
        first = True
        for mt in range(2):
            for tr in range(n_tr):
                last = (mt == 1 and tr == n_tr - 1)
                nc.tensor.matmul(out_ps[:, mt * C:(mt + 1) * C],
                                 X3[:, tr, mt * P:(mt + 1) * P], Y3[:, tr, :],
                                 start=first, stop=last)
                first = False

    xpool = ctx.enter_context(tc.tile_pool(name="xpool", bufs=4))

    with tc.tile_pool(name="build", bufs=1) as build, \
         tc.tile_pool(name="wstate", bufs=2) as wpool, \
         tc.tile_pool(name="mpool", bufs=4) as mpool, \
         tc.tile_pool(name="vpool", bufs=8) as vpool, \
         tc.tile_pool(name="ccdram", bufs=1, space="DRAM") as ccdram:

        Wcur = wpool.tile([P, 2, C], F32R, tag="W")
        WTcur = wpool.tile([P, 2, C], F32R, tag="WT")
        G_sb = build.tile([P, 2, C], F32R)
        G2_sb = build.tile([P, 2, C], F32R)
        G16_sb = build.tile([P, 2, C], F32R)

        for tr in range(2):
            nc.sync.dma_start(Wcur[:, tr, :], pmk[tr * P:(tr + 1) * P, :])

        # ============ phase 1: sigma via repeated squaring (own matrix) ======
        with tc.tile_pool(name="ps1", bufs=2, space="PSUM") as ps1:
            gps = ps1.tile([P, 2 * C], F32, tag="sq")
            prod_mms(gps, Wcur, Wcur)
            nc.scalar.copy(flat(G_sb), gps[:])

            prev = G_sb
            for pw in (2, 4, 8, 16):
                sq = ps1.tile([P, 2 * C], F32, tag="sq", name=f"sq{pw}")
                prod_mms(sq, prev, prev)
                if pw == 2:
                    dst = G2_sb
                elif pw == 16:
                    dst = G16_sb
                else:
                    dst = build.tile([P, 2, C], F32R, tag="gtmp",
                                     name=f"g{pw}", bufs=2)
                if pw in (4, 16):
                    nc.vector.tensor_copy(flat(dst), sq[:])
                else:
                    nc.scalar.copy(flat(dst), sq[:])
                prev = dst

            def matvec(G3, vin, nm):
                vout = vpool.tile([P, 2], F32R, tag="v", name=f"v_{nm}")
                for mt in range(2):
                    vps = ps1.tile([P, 1], F32, tag="vps", bufs=4,
                                   name=f"vp_{nm}_{mt}")
                    for tr in range(2):
                        nc.tensor.matmul(
                            vps[:], G3[:, tr, mt * P:(mt + 1) * P].bitcast(F32),
                            vin[:, tr:tr + 1].bitcast(F32),
                            start=(tr == 0), stop=(tr == 1))
                    nc.scalar.copy(vout[:, mt:mt + 1], vps[:])
                return vout

            v1 = matvec(Wcur, U_sb, "v1")
            m1 = matvec(G2_sb, v1, "m1")
            m2 = matvec(G16_sb, m1, "m2")
            m3 = matvec(G_sb, m2, "m3")

            def dot(va, vb, nm):
                dps = ps1.tile([1, 1], F32, tag="vps", bufs=4, name=f"d_{nm}")
                for tr in range(2):
                    nc.tensor.matmul(dps[:], va[:, tr:tr + 1].bitcast(F32),
                                     vb[:, tr:tr + 1].bitcast(F32),
                                     start=(tr == 0), stop=(tr == 1))
                return dps

            dps0 = dot(v1, m2, "0")
            dps1 = dot(v1, m3, "1")
            dsb = small.tile([1, 3], F32, tag="dsb")
            nc.vector.tensor_copy(dsb[:, 0:1], dps0[:])
            nc.vector.reciprocal(dsb[:, 1:2], dps1[:])
            nc.vector.tensor_mul(dsb[:, 2:3], dsb[:, 0:1], dsb[:, 1:2])
            rsb = small.tile([1, 1], F32, tag="rsb")
            nc.scalar.sqrt(rsb[:], dsb[:, 2:3])
            nc.gpsimd.partition_broadcast(RB[:, 0:1], rsb[:])
            # W0 = A * r (in place), then WT0 = W0^T via PE transpose
            nc.vector.tensor_scalar_mul(Wcur[:], Wcur[:], RB[:, 0:1])
            for tr in range(2):
                for mt in range(2):
                    tps = ps1.tile([P, P], F32, tag="tp")
                    nc.tensor.transpose(
                        tps[:], Wcur[:, tr, mt * P:(mt + 1) * P].bitcast(F32),
                        ID1[:])
                    nc.scalar.copy(WTcur[:, mt, tr * P:(tr + 1) * P], tps[:])

        # ================= phase 2: Bjorck (own matrix) =================
        with tc.tile_pool(name="ps2", bufs=2, space="PSUM") as ps2:
            for it in range(BJORCK_ITERS):
                last = it == BJORCK_ITERS - 1
                Wnxt = None if last else wpool.tile([P, 2, C], F32R, tag="W",
                                                    name=f"W_{it}")
                WTnxt = wpool.tile([P, 2, C], F32R, tag="WT", name=f"WT_{it}")
                gps = ps2.tile([P, 2 * C], F32, tag="g", bufs=2)
                prod_mms(gps, Wcur, Wcur)
                m_sb = mpool.tile([P, 2 * C], F32R, tag="m", name=f"m_{it}")
                nc.vector.scalar_tensor_tensor(
                    m_sb[:], gps[:], -0.5, I15f, op0=ALU.mult, op1=ALU.add)
                m3 = m_sb.rearrange("p (a b) -> p a b", b=C)
                if not last:
                    wps = ps2.tile([P, 2 * C], F32, tag="w", bufs=2)
                    prod_mms(wps, WTcur, m3)
                    nc.scalar.copy(flat(Wnxt), wps[:])
                wtps = ps2.tile([P, 2 * C], F32, tag="wt", bufs=2)
                prod_mms(wtps, m3, WTcur)
                nc.vector.tensor_copy(flat(WTnxt), wtps[:])
                if Wnxt is not None:
                    Wcur = Wnxt
                WTcur = WTnxt

        # ============ AllGather the 5 WT matrices across cores ============
        cc_in = ccdram.tile([1, P * 2 * C], F32R)
        cc_out = ccdram.tile([N_CORES, P * 2 * C], F32R, addr_space="Shared")
        nc.sync.dma_start(cc_in[0].rearrange("(p n) -> p n", p=P), flat(WTcur))
        nc.gpsimd.collective_compute(
            "AllGather", ALU.bypass, ins=[cc_in.opt()], outs=[cc_out.opt()],
            replica_groups=[list(range(N_CORES))])
        for k in range(NK):
            nc.sync.dma_start(flat(WTfin[:, k]),
                              cc_out[k].rearrange("(p n) -> p n", p=P))

        # ================= phase 3: weight assembly =================
        PQ_sb = build.tile([P, 4, 2, C], F32R)
        IP_sb = build.tile([P, 4, 2, C], F32R)
        b1T_sb = build.tile([P, 2, 2, 2, C], F32R)
        b2_sb = build.tile([P, 2, 2, 2, C], F32R)
        p3_sb = build.tile([P, 9, 2, C], F32R)

        with tc.tile_pool(name="ps3", bufs=4, space="PSUM") as ps3:
            for i in range(4):
                qps = ps3.tile([P, 2 * C], F32, tag="as", name=f"q_{i}")
                prod_mms(qps, WTfin[:, i + 1], WTfin[:, i + 1], n_tr=1)
                nc.scalar.copy(flat(PQ_sb[:, i]), qps[:])
                nc.vector.tensor_sub(flat(IP_sb[:, i]), flat(I10), flat(PQ_sb[:, i]))

            def symprod(dst3, X3, Y3, nm):
                bps = ps3.tile([P, 2 * C], F32, tag="as", name=f"bp_{nm}")
                prod_mms(bps, X3, Y3)
                nc.scalar.copy(flat(dst3), bps[:])

            for i1 in range(2):
                for j1 in range(2):
                    symprod(b1T_sb[:, i1, j1],
                            IP_sb[:, 1] if j1 else PQ_sb[:, 1],
                            IP_sb[:, 0] if i1 else PQ_sb[:, 0], f"b1T{i1}{j1}")
            for i2 in range(2):
                for j2 in range(2):
                    symprod(b2_sb[:, i2, j2],
                            IP_sb[:, 2] if i2 else PQ_sb[:, 2],
                            IP_sb[:, 3] if j2 else PQ_sb[:, 3], f"b2{i2}{j2}")

            for i in range(3):
                for j in range(3):
                    terms = [(i1, j1) for i1 in range(2) for j1 in range(2)
                             if 0 <= i - i1 < 2 and 0 <= j - j1 < 2]
                    pps = ps3.tile([P, 2 * C], F32, tag="as", name=f"p3_{i}{j}")
                    nmm = len(terms) * 4
                    idx = 0
                    for mt in range(2):
                        for (i1, j1) in terms:
                            for tr in range(2):
                                idx += 1
                                nc.tensor.matmul(
                                    pps[:, mt * C:(mt + 1) * C],
                                    b1T_sb[:, i1, j1, tr, mt * P:(mt + 1) * P],
                                    b2_sb[:, i - i1, j - j1, tr, :],
                                    start=(idx == 1), stop=(idx == nmm))
                    nc.vector.tensor_copy(flat(p3_sb[:, 3 * i + j]), pps[:])

            for i in range(3):
                for j in range(3):
                    eps_ = ps3.tile([P, 2 * C], F32, tag="as", name=f"pe_{i}{j}")
                    prod_mms(eps_, WTfin[:, 0], p3_sb[:, 3 * i + j])
                    nc.scalar.copy(flat(Wc_sb[:, 3 * i + j]), eps_[:])

    # ================= phase 4: conv =================
    with tc.tile_pool(name="opool", bufs=3) as opool, \
         tc.tile_pool(name="psC", bufs=6, space="PSUM") as psC:
        for b in range(B_CORE):
            xp = []
            for tr in range(2):
                xpt = xpool.tile([P, PH, PH], F32R, tag="xp", name=f"xp_{b}_{tr}")
                nc.sync.dma_start(xpt[:, 1:65, 1:65], xs[b, tr * P:(tr + 1) * P, :, :])
                nc.vector.tensor_copy(xpt[:, 1:65, 0:1], xpt[:, 1:65, 64:65])
                nc.vector.tensor_copy(xpt[:, 1:65, 65:66], xpt[:, 1:65, 1:2])
                nc.vector.tensor_copy(xpt[:, 0:1, :], xpt[:, 64:65, :])
                nc.vector.tensor_copy(xpt[:, 65:66, :], xpt[:, 1:2, :])
                xp.append(xpt)
            for mt in range(2):
                osb = opool.tile([P, NPIX], F32, tag="osb", name=f"osb_{b}_{mt}")
                for pb in range(8):
                    ops = psC.tile([P, 512], F32, tag="o")
                    idx = 0
                    for kh in range(3):
                        for kw in range(3):
                            for tr in range(2):
                                nc.tensor.matmul(
                                    ops[:],
                                    Wc_sb[:, 3 * kw + kh, tr, mt * P:(mt + 1) * P],
                                    xp[tr][:, pb * 8 + kh:pb * 8 + kh + 8,
                                           kw:kw + 64],
                                    start=(idx == 0), stop=(idx == 17))
                                idx += 1
                    nc.scalar.activation(
                        osb[:, pb * 512:(pb + 1) * 512], ops[:], ACTF.Identity,
                        bias=bias_sb[:, mt, :], scale=1.0)
                    nc.sync.dma_start(
                        out_ap[b, mt * P:(mt + 1) * P, pb * 8:(pb + 1) * 8, :],
                        osb[:, pb * 512:(pb + 1) * 512].rearrange(
                            "p (h w) -> p h w", w=H))


def build_program():
    from contextlib import ExitStack
    nc = bacc.Bacc("TRN2", target_bir_lowering=False, debug=False,
                   enable_asserts=False, num_devices=N_CORES)
    xs = nc.dram_tensor("xs", [B_CORE, C, H, H], F32R, kind="ExternalInput").ap()
    pmk = nc.dram_tensor("pmk", [C, C], F32R, kind="ExternalInput").ap()
    u0k = nc.dram_tensor("u0k", [C, 1], F32R, kind="ExternalInput").ap()
    bias = nc.dram_tensor("bias", [C], F32, kind="ExternalInput").ap()
    out = nc.dram_tensor("out", [B_CORE, C, H, H], F32, kind="ExternalOutput").ap()
    with tile.TileContext(nc) as tc:
        with ExitStack() as ctx:
            build_body(tc, out, xs, pmk, u0k, bias, ctx)
    nc.compile()
    return nc


_cached_nc = None


def make_in_maps(x, pm, u0, b):
    in_maps = []
    for i in range(N_CORES):
        k = i if i < NK else i - NK
        in_maps.append({
            "xs": np.ascontiguousarray(x[i * B_CORE:(i + 1) * B_CORE]),
            "pmk": np.ascontiguousarray(pm[k]),
            "u0k": np.ascontiguousarray(u0[k]),
            "bias": np.ascontiguousarray(b),
        })
    return in_maps


def kernel(x, param_matrices, init_u, bias):
    global _cached_nc
    if _cached_nc is None:
        _cached_nc = build_program()
    nc = _cached_nc
    x = np.ascontiguousarray(np.asarray(x, dtype=np.float32))
    pm = np.ascontiguousarray(np.asarray(param_matrices, dtype=np.float32))
    u0 = np.ascontiguousarray(np.asarray(init_u, dtype=np.float32))
    b = np.ascontiguousarray(np.asarray(bias, dtype=np.float32))
    in_maps = make_in_maps(x, pm, u0, b)
    res = run_bass_kernel_spmd(nc, in_maps, core_ids=list(range(N_CORES)))
    return np.concatenate([r["out"] for r in res.results], axis=0)


if __name__ == "__main__":
    import reference
    inputs = {k: np.asarray(v) for k, v in reference.setup_inputs().items()}
    out = kernel(**inputs)
    print(out.shape, out.dtype)


# revision 15
# speedup vs baseline: 1.3417x; 1.0026x over previous
"""BCOP (block-convolution orthogonal parameterization) forward on 8 TRN2 cores.

Math (validated vs reference in fp32 numpy):
  - power iteration via repeated squaring: G = A^T A; with v1 = A^T u0,
    d0 = v1.(G^18 v1), d1 = v1.(G^19 v1) reproduce the reference's
    normalized-power-iteration sigma: s = sqrt(d1/d0); G^18 v1 = G16@(G2@v1).
  - W0 = A/s; 20 Bjorck iters maintaining W and WT = W^T:
      G = W^T W  (lhsT=W,rhs=W);  M = 1.5 I - 0.5 G  (symmetric)
      W' = W M   (lhsT=WT,rhs=M); WT' = M WT (lhsT=M,rhs=WT)
  - downstream needs only WT = ortho^T:
      Z^T_i = WT[i+1] with rows>=128 zeroed -> PQ_i = matmul(lhsT=Z^T,rhs=Z^T)
      b1T[i1,j1] = (block_orth(PQ0,PQ1)[i1,j1])^T and b2 = block_orth(PQ2,PQ3),
      all products of symmetric matrices -> no transposes needed.
      p3[i,j] = sum b1[i1,j1] @ b2[i-i1,j-j1] = matmul(lhsT=b1T[..], rhs=b2[..])
      p_e[i,j] = H @ p3[i,j] = matmul(lhsT=WT[0], rhs=p3[i,j])
  - conv tap (kh,kw) uses stationary lhsT[ci,co] = p_e[kw,kh]; x circularly
    padded to 66x66 in SBUF; 18 accumulating matmuls (9 taps x 2 ci-tiles)
    per [128co x 512px] PSUM tile; bias added on the PSUM->SBUF evacuation.

Parallelization: the per-matrix weight construction (power iteration + Bjorck)
is INDEPENDENT across the 5 parameter matrices, so it is sharded across cores
via the inputs (core i receives matrix i mod 5 only), then an 8-rank AllGather
shares the 5 WT results; assembly + conv then run replicated. The conv is
data-parallel over batch (4 images per core).

PSUM discipline: every accumulation group owns a whole bank; a [128,512] tile
holds both 128-row output halves of a 256x256 product as ONE group.
"""

import numpy as np

import concourse.bass as bass
import concourse.mybir as mybir
import concourse.tile as tile
from concourse import bacc
from concourse.bass_utils import run_bass_kernel_spmd

P = 128
C = 256
NK = 5
N_CORES = 8
B_TOTAL = 32
B_CORE = B_TOTAL // N_CORES
H = 64
PH = 66
NPIX = H * H
BJORCK_ITERS = 20

F32 = mybir.dt.float32
F32R = mybir.dt.float32r
ALU = mybir.AluOpType
ACTF = mybir.ActivationFunctionType


def build_body(tc, out_ap, xs, pmk, u0k, bias_ap, ctx):
    nc = tc.nc
    from concourse.masks import make_identity

    persist = ctx.enter_context(tc.tile_pool(name="persist", bufs=1))
    small = ctx.enter_context(tc.tile_pool(name="small", bufs=3))

    U_sb = persist.tile([P, 2, 1], F32R)
    for tr in range(2):
        nc.sync.dma_start(U_sb[:, tr, :], u0k[tr * P:(tr + 1) * P, :])
    bias_sb = persist.tile([P, 2, 1], F32)
    for mt in range(2):
        nc.sync.dma_start(bias_sb[:, mt, :], bias_ap[mt * P:(mt + 1) * P].unsqueeze(1))

    # ---- constants ----
    ID1 = persist.tile([P, P], F32)
    make_identity(nc, ID1)
    I15 = persist.tile([P, 2, C], F32)
    I10 = persist.tile([P, 2, C], F32)
    nc.vector.memset(I15[:], 0.0)
    nc.vector.memset(I10[:], 0.0)
    for mt in range(2):
        nc.scalar.mul(I15[:, mt, mt * P:(mt + 1) * P], ID1[:], 1.5)
        nc.scalar.mul(I10[:, mt, mt * P:(mt + 1) * P], ID1[:], 1.0)
    I15f = I15.rearrange("p a b -> p (a b)")

    RB = persist.tile([P, 1], F32)            # broadcast 1/s (own matrix)
    Wc_sb = persist.tile([P, 9, 2, C], F32R)  # final conv lhsT tiles, slot (i,j)
    WTfin = persist.tile([P, NK, 2, C], F32R)  # gathered ortho^T, all matrices

    def flat(ap3):
        return ap3.rearrange("p a b -> p (a b)")

    def prod_mms(out_ps, X3, Y3, n_tr=2):
        """dst[mt] += X[tr][:, mt]^T @ Y[tr]; ONE accumulation group per bank."""
        first = True
        for mt in range(2):
            for tr in range(n_tr):
                last = (mt == 1 and tr == n_tr - 1)
                nc.tensor.matmul(out_ps[:, mt * C:(mt + 1) * C],
                                 X3[:, tr, mt * P:(mt + 1) * P], Y3[:, tr, :],
                                 start=first, stop=last)
                first = False

    xpool = ctx.enter_context(tc.tile_pool(name="xpool", bufs=4))

    with tc.tile_pool(name="build", bufs=1) as build, \
         tc.tile_pool(name="wstate", bufs=2) as wpool, \
         tc.tile_pool(name="mpool", bufs=4) as mpool, \
         tc.tile_pool(name="vpool", bufs=8) as vpool, \
         tc.tile_pool(name="ccdram", bufs=1, space="DRAM") as ccdram:

        Wcur = wpool.tile([P, 2, C], F32R, tag="W")
        WTcur = wpool.tile([P, 2, C], F32R, tag="WT")
        G_sb = build.tile([P, 2, C], F32R)
        G2_sb = build.tile([P, 2, C], F32R)
        G16_sb = build.tile([P, 2, C], F32R)

        for tr in range(2):
            nc.sync.dma_start(Wcur[:, tr, :], pmk[tr * P:(tr + 1) * P, :])

        # ============ phase 1: sigma via repeated squaring (own matrix) ======
        with tc.tile_pool(name="ps1", bufs=2, space="PSUM") as ps1:
            gps = ps1.tile([P, 2 * C], F32, tag="sq")
            prod_mms(gps, Wcur, Wcur)
            nc.scalar.copy(flat(G_sb), gps[:])

            prev = G_sb
            for pw in (2, 4, 8, 16):
                sq = ps1.tile([P, 2 * C], F32, tag="sq", name=f"sq{pw}")
                prod_mms(sq, prev, prev)
                if pw == 2:
                    dst = G2_sb
                elif pw == 16:
                    dst = G16_sb
                else:
                    dst = build.tile([P, 2, C], F32R, tag="gtmp",
                                     name=f"g{pw}", bufs=2)
                if pw in (4, 16):
                    nc.vector.tensor_copy(flat(dst), sq[:])
                else:
                    nc.scalar.copy(flat(dst), sq[:])
                prev = dst

            def matvec(G3, vin, nm):
                vout = vpool.tile([P, 2], F32R, tag="v", name=f"v_{nm}")
                for mt in range(2):
                    vps = ps1.tile([P, 1], F32, tag="vps", bufs=4,
                                   name=f"vp_{nm}_{mt}")
                    for tr in range(2):
                        nc.tensor.matmul(
                            vps[:], G3[:, tr, mt * P:(mt + 1) * P].bitcast(F32),
                            vin[:, tr:tr + 1].bitcast(F32),
                            start=(tr == 0), stop=(tr == 1))
                    nc.scalar.copy(vout[:, mt:mt + 1], vps[:])
                return vout

            v1 = matvec(Wcur, U_sb, "v1")
            m1 = matvec(G2_sb, v1, "m1")
            m2 = matvec(G16_sb, m1, "m2")
            m3 = matvec(G_sb, m2, "m3")

            def dot(va, vb, nm):
                dps = ps1.tile([1, 1], F32, tag="vps", bufs=4, name=f"d_{nm}")
                for tr in range(2):
                    nc.tensor.matmul(dps[:], va[:, tr:tr + 1].bitcast(F32),
                                     vb[:, tr:tr + 1].bitcast(F32),
                                     start=(tr == 0), stop=(tr == 1))
                return dps

            dps0 = dot(v1, m2, "0")
            dps1 = dot(v1, m3, "1")
            dsb = small.tile([1, 3], F32, tag="dsb")
            nc.vector.tensor_copy(dsb[:, 0:1], dps0[:])
            nc.vector.reciprocal(dsb[:, 1:2], dps1[:])
            nc.vector.tensor_mul(dsb[:, 2:3], dsb[:, 0:1], dsb[:, 1:2])
            rsb = small.tile([1, 1], F32, tag="rsb")
            nc.scalar.sqrt(rsb[:], dsb[:, 2:3])
            nc.gpsimd.partition_broadcast(RB[:, 0:1], rsb[:])
            # W0 = A * r (in place), then WT0 = W0^T via PE transpose
            nc.vector.tensor_scalar_mul(Wcur[:], Wcur[:], RB[:, 0:1])
            for tr in range(2):
                for mt in range(2):
                    tps = ps1.tile([P, P], F32, tag="tp")
                    nc.tensor.transpose(
                        tps[:], Wcur[:, tr, mt * P:(mt + 1) * P].bitcast(F32),
                        ID1[:])
                    nc.scalar.copy(WTcur[:, mt, tr * P:(tr + 1) * P], tps[:])

        # ================= phase 2: Bjorck (own matrix) =================
        with tc.tile_pool(name="ps2", bufs=2, space="PSUM") as ps2:
            for it in range(BJORCK_ITERS):
                last = it == BJORCK_ITERS - 1
                Wnxt = None if last else wpool.tile([P, 2, C], F32R, tag="W",
                                                    name=f"W_{it}")
                WTnxt = wpool.tile([P, 2, C], F32R, tag="WT", name=f"WT_{it}")
                gps = ps2.tile([P, 2 * C], F32, tag="g", bufs=2)
                prod_mms(gps, Wcur, Wcur)
                m_sb = mpool.tile([P, 2 * C], F32R, tag="m", name=f"m_{it}")
                nc.vector.scalar_tensor_tensor(
                    m_sb[:], gps[:], -0.5, I15f, op0=ALU.mult, op1=ALU.add)
                m3 = m_sb.rearrange("p (a b) -> p a b", b=C)
                if not last:
                    wps = ps2.tile([P, 2 * C], F32, tag="w", bufs=2)
                    prod_mms(wps, WTcur, m3)
                    nc.scalar.copy(flat(Wnxt), wps[:])
                wtps = ps2.tile([P, 2 * C], F32, tag="wt", bufs=2)
                prod_mms(wtps, m3, WTcur)
                nc.vector.tensor_copy(flat(WTnxt), wtps[:])
                if Wnxt is not None:
                    Wcur = Wnxt
                WTcur = WTnxt

        # ============ AllGather the 5 WT matrices across cores ============
        cc_in = ccdram.tile([1, P * 2 * C], F32R)
        cc_out = ccdram.tile([N_CORES, P * 2 * C], F32R, addr_space="Shared")
        nc.sync.dma_start(cc_in[0].rearrange("(p n) -> p n", p=P), flat(WTcur))
        nc.gpsimd.collective_compute(
            "AllGather", ALU.bypass, ins=[cc_in.opt()], outs=[cc_out.opt()],
            replica_groups=[list(range(N_CORES))])
        for k in range(NK):
            nc.sync.dma_start(flat(WTfin[:, k]),
                              cc_out[k].rearrange("(p n) -> p n", p=P))



        # ================= phase 3: weight assembly =================
        PQ_sb = build.tile([P, 4, 2, C], F32R)
        IP_sb = build.tile([P, 4, 2, C], F32R)
        b1T_sb = build.tile([P, 2, 2, 2, C], F32R)
        b2_sb = build.tile([P, 2, 2, 2, C], F32R)
        p3_sb = build.tile([P, 9, 2, C], F32R)

        with tc.tile_pool(name="ps3", bufs=4, space="PSUM") as ps3:
            # Keep the PE drawing steady power through the AllGather wait:
            # the power governor clamps the clock to 13/16 for ~300us when
            # all 8 cores jump from idle to dense matmul at the same moment
            # (post-barrier). ~55us of dummy matmuls bridge the gap; a copy
            # into a live SBUF tile anchors them against DCE.
            warm_sb = small.tile([1, 1], F32, tag="wmsb")
            for wj in range(13):
                warm_ps = ps3.tile([P, 2 * C], F32, tag="wm", bufs=2,
                                   name=f"warm_{wj}")
                for wi in range(20):
                    nc.tensor.matmul(warm_ps[:], WTcur[:, 0, 0:P], flat(WTcur),
                                     start=(wi == 0), stop=(wi == 19))
                nc.vector.tensor_copy(warm_sb[:], warm_ps[0:1, 0:1])
            for i in range(4):
                qps = ps3.tile([P, 2 * C], F32, tag="as", name=f"q_{i}")
                prod_mms(qps, WTfin[:, i + 1], WTfin[:, i + 1], n_tr=1)
                nc.scalar.copy(flat(PQ_sb[:, i]), qps[:])
                nc.vector.tensor_sub(flat(IP_sb[:, i]), flat(I10), flat(PQ_sb[:, i]))

            def symprod(dst3, X3, Y3, nm):
                bps = ps3.tile([P, 2 * C], F32, tag="as", name=f"bp_{nm}")
                prod_mms(bps, X3, Y3)
                nc.scalar.copy(flat(dst3), bps[:])

            for i1 in range(2):
                for j1 in range(2):
                    symprod(b1T_sb[:, i1, j1],
                            IP_sb[:, 1] if j1 else PQ_sb[:, 1],
                            IP_sb[:, 0] if i1 else PQ_sb[:, 0], f"b1T{i1}{j1}")
            for i2 in range(2):
                for j2 in range(2):
                    symprod(b2_sb[:, i2, j2],
                            IP_sb[:, 2] if i2 else PQ_sb[:, 2],
                            IP_sb[:, 3] if j2 else PQ_sb[:, 3], f"b2{i2}{j2}")

            for i in range(3):
                for j in range(3):
                    terms = [(i1, j1) for i1 in range(2) for j1 in range(2)
                             if 0 <= i - i1 < 2 and 0 <= j - j1 < 2]
                    pps = ps3.tile([P, 2 * C], F32, tag="as", name=f"p3_{i}{j}")
                    nmm = len(terms) * 4
                    idx = 0
                    for mt in range(2):
                        for (i1, j1) in terms:
                            for tr in range(2):
                                idx += 1
                                nc.tensor.matmul(
                                    pps[:, mt * C:(mt + 1) * C],
                                    b1T_sb[:, i1, j1, tr, mt * P:(mt + 1) * P],
                                    b2_sb[:, i - i1, j - j1, tr, :],
                                    start=(idx == 1), stop=(idx == nmm))
                    nc.vector.tensor_copy(flat(p3_sb[:, 3 * i + j]), pps[:])

            for i in range(3):
                for j in range(3):
                    eps_ = ps3.tile([P, 2 * C], F32, tag="as", name=f"pe_{i}{j}")
                    prod_mms(eps_, WTfin[:, 0], p3_sb[:, 3 * i + j])
                    nc.scalar.copy(flat(Wc_sb[:, 3 * i + j]), eps_[:])

    # ================= phase 4: conv =================
    with tc.tile_pool(name="opool", bufs=3) as opool, \
         tc.tile_pool(name="psC", bufs=6, space="PSUM") as psC:
        for b in range(B_CORE):
            xp = []
            for tr in range(2):
                xpt = xpool.tile([P, PH, PH], F32R, tag="xp", name=f"xp_{b}_{tr}")
                nc.sync.dma_start(xpt[:, 1:65, 1:65], xs[b, tr * P:(tr + 1) * P, :, :])
                nc.vector.tensor_copy(xpt[:, 1:65, 0:1], xpt[:, 1:65, 64:65])
                nc.vector.tensor_copy(xpt[:, 1:65, 65:66], xpt[:, 1:65, 1:2])
                nc.vector.tensor_copy(xpt[:, 0:1, :], xpt[:, 64:65, :])
                nc.vector.tensor_copy(xpt[:, 65:66, :], xpt[:, 1:2, :])
                xp.append(xpt)
            for mt in range(2):
                osb = opool.tile([P, NPIX], F32, tag="osb", name=f"osb_{b}_{mt}")
                for pb in range(8):
                    ops = psC.tile([P, 512], F32, tag="o")
                    idx = 0
                    for kh in range(3):
                        for kw in range(3):
                            for tr in range(2):
                                nc.tensor.matmul(
                                    ops[:],
                                    Wc_sb[:, 3 * kw + kh, tr, mt * P:(mt + 1) * P],
                                    xp[tr][:, pb * 8 + kh:pb * 8 + kh + 8,
                                           kw:kw + 64],
                                    start=(idx == 0), stop=(idx == 17))
                                idx += 1
                    nc.scalar.activation(
                        osb[:, pb * 512:(pb + 1) * 512], ops[:], ACTF.Identity,
                        bias=bias_sb[:, mt, :], scale=1.0)
                    nc.sync.dma_start(
                        out_ap[b, mt * P:(mt + 1) * P, pb * 8:(pb + 1) * 8, :],
                        osb[:, pb * 512:(pb + 1) * 512].rearrange(
                            "p (h w) -> p h w", w=H))


def build_program():
    from contextlib import ExitStack
    nc = bacc.Bacc("TRN2", target_bir_lowering=False, debug=False,
                   enable_asserts=False, num_devices=N_CORES)
    xs = nc.dram_tensor("xs", [B_CORE, C, H, H], F32R, kind="ExternalInput").ap()
    pmk = nc.dram_tensor("pmk", [C, C], F32R, kind="ExternalInput").ap()
    u0k = nc.dram_tensor("u0k", [C, 1], F32R, kind="ExternalInput").ap()
    bias = nc.dram_tensor("bias", [C], F32, kind="ExternalInput").ap()
    out = nc.dram_tensor("out", [B_CORE, C, H, H], F32, kind="ExternalOutput").ap()
    with tile.TileContext(nc) as tc:
        with ExitStack() as ctx:
            build_body(tc, out, xs, pmk, u0k, bias, ctx)
    nc.compile()
    return nc


_cached_nc = None


def make_in_maps(x, pm, u0, b):
    in_maps = []
    for i in range(N_CORES):
        k = i if i < NK else i - NK
        in_maps.append({
            "xs": np.ascontiguousarray(x[i * B_CORE:(i + 1) * B_CORE]),
            "pmk": np.ascontiguousarray(pm[k]),
            "u0k": np.ascontiguousarray(u0[k]),
            "bias": np.ascontiguousarray(b),
        })
    return in_maps


def kernel(x, param_matrices, init_u, bias):
    global _cached_nc
    if _cached_nc is None:
        _cached_nc = build_program()
    nc = _cached_nc
    x = np.ascontiguousarray(np.asarray(x, dtype=np.float32))
    pm = np.ascontiguousarray(np.asarray(param_matrices, dtype=np.float32))
    u0 = np.ascontiguousarray(np.asarray(init_u, dtype=np.float32))
    b = np.ascontiguousarray(np.asarray(bias, dtype=np.float32))
    in_maps = make_in_maps(x, pm, u0, b)
    res = run_bass_kernel_spmd(nc, in_maps, core_ids=list(range(N_CORES)))
    return np.concatenate([r["out"] for r in res.results], axis=0)


if __name__ == "__main__":
    import reference
    inputs = {k: np.asarray(v) for k, v in reference.setup_inputs().items()}
    out = kernel(**inputs)
    print(out.shape, out.dtype)


# revision 16
# speedup vs baseline: 1.3444x; 1.0020x over previous
"""BCOP (block-convolution orthogonal parameterization) forward on 8 TRN2 cores.

Math (validated vs reference in fp32 numpy):
  - power iteration via repeated squaring: G = A^T A; with v1 = A^T u0,
    d0 = v1.(G^18 v1), d1 = v1.(G^19 v1) reproduce the reference's
    normalized-power-iteration sigma: s = sqrt(d1/d0); G^18 v1 = G16@(G2@v1).
  - W0 = A/s; 20 Bjorck iters maintaining W and WT = W^T:
      G = W^T W  (lhsT=W,rhs=W);  M = 1.5 I - 0.5 G  (symmetric)
      W' = W M   (lhsT=WT,rhs=M); WT' = M WT (lhsT=M,rhs=WT)
  - downstream needs only WT = ortho^T:
      Z^T_i = WT[i+1] with rows>=128 zeroed -> PQ_i = matmul(lhsT=Z^T,rhs=Z^T)
      b1T[i1,j1] = (block_orth(PQ0,PQ1)[i1,j1])^T and b2 = block_orth(PQ2,PQ3),
      all products of symmetric matrices -> no transposes needed.
      p3[i,j] = sum b1[i1,j1] @ b2[i-i1,j-j1] = matmul(lhsT=b1T[..], rhs=b2[..])
      p_e[i,j] = H @ p3[i,j] = matmul(lhsT=WT[0], rhs=p3[i,j])
  - conv tap (kh,kw) uses stationary lhsT[ci,co] = p_e[kw,kh]; x circularly
    padded to 66x66 in SBUF; 18 accumulating matmuls (9 taps x 2 ci-tiles)
    per [128co x 512px] PSUM tile; bias added on the PSUM->SBUF evacuation.

Parallelization: the per-matrix weight construction (power iteration + Bjorck)
is INDEPENDENT across the 5 parameter matrices, so it is sharded across cores
via the inputs (core i receives matrix i mod 5 only), then an 8-rank AllGather
shares the 5 WT results; assembly + conv then run replicated. The conv is
data-parallel over batch (4 images per core).

PSUM discipline: every accumulation group owns a whole bank; a [128,512] tile
holds both 128-row output halves of a 256x256 product as ONE group.
"""

import numpy as np

import concourse.bass as bass
import concourse.mybir as mybir
import concourse.tile as tile
from concourse import bacc
from concourse.bass_utils import run_bass_kernel_spmd

P = 128
C = 256
NK = 5
N_CORES = 8
B_TOTAL = 32
B_CORE = B_TOTAL // N_CORES
H = 64
PH = 66
NPIX = H * H
BJORCK_ITERS = 20

F32 = mybir.dt.float32
F32R = mybir.dt.float32r
ALU = mybir.AluOpType
ACTF = mybir.ActivationFunctionType


def build_body(tc, out_ap, xs, pmk, u0k, bias_ap, ctx):
    nc = tc.nc
    from concourse.masks import make_identity

    persist = ctx.enter_context(tc.tile_pool(name="persist", bufs=1))
    small = ctx.enter_context(tc.tile_pool(name="small", bufs=3))

    U_sb = persist.tile([P, 2, 1], F32R)
    for tr in range(2):
        nc.sync.dma_start(U_sb[:, tr, :], u0k[tr * P:(tr + 1) * P, :])
    bias_sb = persist.tile([P, 2, 1], F32)
    for mt in range(2):
        nc.sync.dma_start(bias_sb[:, mt, :], bias_ap[mt * P:(mt + 1) * P].unsqueeze(1))

    # ---- constants ----
    ID1 = persist.tile([P, P], F32)
    make_identity(nc, ID1)
    I15 = persist.tile([P, 2, C], F32)
    I10 = persist.tile([P, 2, C], F32)
    nc.vector.memset(I15[:], 0.0)
    nc.vector.memset(I10[:], 0.0)
    for mt in range(2):
        nc.scalar.mul(I15[:, mt, mt * P:(mt + 1) * P], ID1[:], 1.5)
        nc.scalar.mul(I10[:, mt, mt * P:(mt + 1) * P], ID1[:], 1.0)
    I15f = I15.rearrange("p a b -> p (a b)")

    RB = persist.tile([P, 1], F32)            # broadcast 1/s (own matrix)
    Wc_sb = persist.tile([P, 9, 2, C], F32R)  # final conv lhsT tiles, slot (i,j)
    WTfin = persist.tile([P, NK, 2, C], F32R)  # gathered ortho^T, all matrices

    def flat(ap3):
        return ap3.rearrange("p a b -> p (a b)")

    def prod_mms(out_ps, X3, Y3, n_tr=2):
        """dst[mt] += X[tr][:, mt]^T @ Y[tr]; ONE accumulation group per bank."""
        first = True
        for mt in range(2):
            for tr in range(n_tr):
                last = (mt == 1 and tr == n_tr - 1)
                nc.tensor.matmul(out_ps[:, mt * C:(mt + 1) * C],
                                 X3[:, tr, mt * P:(mt + 1) * P], Y3[:, tr, :],
                                 start=first, stop=last)
                first = False

    xpool = ctx.enter_context(tc.tile_pool(name="xpool", bufs=4))

    with tc.tile_pool(name="build", bufs=1) as build, \
         tc.tile_pool(name="wstate", bufs=2) as wpool, \
         tc.tile_pool(name="mpool", bufs=4) as mpool, \
         tc.tile_pool(name="vpool", bufs=8) as vpool, \
         tc.tile_pool(name="ccdram", bufs=1, space="DRAM") as ccdram:

        Wcur = wpool.tile([P, 2, C], F32R, tag="W")
        WTcur = wpool.tile([P, 2, C], F32R, tag="WT")
        G_sb = build.tile([P, 2, C], F32R)
        G2_sb = build.tile([P, 2, C], F32R)
        G16_sb = build.tile([P, 2, C], F32R)

        for tr in range(2):
            nc.sync.dma_start(Wcur[:, tr, :], pmk[tr * P:(tr + 1) * P, :])

        # ============ phase 1: sigma via repeated squaring (own matrix) ======
        with tc.tile_pool(name="ps1", bufs=2, space="PSUM") as ps1:
            gps = ps1.tile([P, 2 * C], F32, tag="sq")
            prod_mms(gps, Wcur, Wcur)
            nc.scalar.copy(flat(G_sb), gps[:])

            prev = G_sb
            for pw in (2, 4, 8, 16):
                sq = ps1.tile([P, 2 * C], F32, tag="sq", name=f"sq{pw}")
                prod_mms(sq, prev, prev)
                if pw == 2:
                    dst = G2_sb
                elif pw == 16:
                    dst = G16_sb
                else:
                    dst = build.tile([P, 2, C], F32R, tag="gtmp",
                                     name=f"g{pw}", bufs=2)
                if pw in (4, 16):
                    nc.vector.tensor_copy(flat(dst), sq[:])
                else:
                    nc.scalar.copy(flat(dst), sq[:])
                prev = dst

            def matvec(G3, vin, nm):
                vout = vpool.tile([P, 2], F32R, tag="v", name=f"v_{nm}")
                for mt in range(2):
                    vps = ps1.tile([P, 1], F32, tag="vps", bufs=4,
                                   name=f"vp_{nm}_{mt}")
                    for tr in range(2):
                        nc.tensor.matmul(
                            vps[:], G3[:, tr, mt * P:(mt + 1) * P].bitcast(F32),
                            vin[:, tr:tr + 1].bitcast(F32),
                            start=(tr == 0), stop=(tr == 1))
                    nc.scalar.copy(vout[:, mt:mt + 1], vps[:])
                return vout

            v1 = matvec(Wcur, U_sb, "v1")
            m1 = matvec(G2_sb, v1, "m1")
            m2 = matvec(G16_sb, m1, "m2")
            m3 = matvec(G_sb, m2, "m3")

            def dot(va, vb, nm):
                dps = ps1.tile([1, 1], F32, tag="vps", bufs=4, name=f"d_{nm}")
                for tr in range(2):
                    nc.tensor.matmul(dps[:], va[:, tr:tr + 1].bitcast(F32),
                                     vb[:, tr:tr + 1].bitcast(F32),
                                     start=(tr == 0), stop=(tr == 1))
                return dps

            dps0 = dot(v1, m2, "0")
            dps1 = dot(v1, m3, "1")
            dsb = small.tile([1, 3], F32, tag="dsb")
            nc.vector.tensor_copy(dsb[:, 0:1], dps0[:])
            nc.vector.reciprocal(dsb[:, 1:2], dps1[:])
            nc.vector.tensor_mul(dsb[:, 2:3], dsb[:, 0:1], dsb[:, 1:2])
            rsb = small.tile([1, 1], F32, tag="rsb")
            nc.scalar.sqrt(rsb[:], dsb[:, 2:3])
            nc.gpsimd.partition_broadcast(RB[:, 0:1], rsb[:])
            # W0 = A * r (in place), then WT0 = W0^T via PE transpose
            nc.vector.tensor_scalar_mul(Wcur[:], Wcur[:], RB[:, 0:1])
            for tr in range(2):
                for mt in range(2):
                    tps = ps1.tile([P, P], F32, tag="tp")
                    nc.tensor.transpose(
                        tps[:], Wcur[:, tr, mt * P:(mt + 1) * P].bitcast(F32),
                        ID1[:])
                    nc.scalar.copy(WTcur[:, mt, tr * P:(tr + 1) * P], tps[:])

        # ================= phase 2: Bjorck (own matrix) =================
        with tc.tile_pool(name="ps2", bufs=2, space="PSUM") as ps2:
            for it in range(BJORCK_ITERS):
                last = it == BJORCK_ITERS - 1
                Wnxt = None if last else wpool.tile([P, 2, C], F32R, tag="W",
                                                    name=f"W_{it}")
                WTnxt = wpool.tile([P, 2, C], F32R, tag="WT", name=f"WT_{it}")
                gps = ps2.tile([P, 2 * C], F32, tag="g", bufs=2)
                prod_mms(gps, Wcur, Wcur)
                m_sb = mpool.tile([P, 2 * C], F32R, tag="m", name=f"m_{it}")
                nc.vector.scalar_tensor_tensor(
                    m_sb[:], gps[:], -0.5, I15f, op0=ALU.mult, op1=ALU.add)
                m3 = m_sb.rearrange("p (a b) -> p a b", b=C)
                if not last:
                    wps = ps2.tile([P, 2 * C], F32, tag="w", bufs=2)
                    prod_mms(wps, WTcur, m3)
                    nc.scalar.copy(flat(Wnxt), wps[:])
                wtps = ps2.tile([P, 2 * C], F32, tag="wt", bufs=2)
                prod_mms(wtps, m3, WTcur)
                nc.vector.tensor_copy(flat(WTnxt), wtps[:])
                if Wnxt is not None:
                    Wcur = Wnxt
                WTcur = WTnxt

        # ============ AllGather the needed WT halves across cores ============
        # Assembly reads only row-tile 0 of WT[1..4] (the masked projections)
        # but both row-tiles of WT[0] (H). Cores 0-4 contribute their tr=0
        # half; core 5 (a k=0 duplicate) contributes k=0's tr=1 half via a
        # partition-id-predicated DMA. Halves the AllGather payload.
        cc_in = ccdram.tile([1, P * C], F32R)
        cc_out = ccdram.tile([N_CORES, P * C], F32R, addr_space="Shared")
        pid = nc.sync.partition_id()
        nc.sync.dma_start(cc_in[0].rearrange("(p n) -> p n", p=P),
                          WTcur[:, 0, :], cond=(pid != 5))
        nc.sync.dma_start(cc_in[0].rearrange("(p n) -> p n", p=P),
                          WTcur[:, 1, :], cond=(pid == 5))
        nc.gpsimd.collective_compute(
            "AllGather", ALU.bypass, ins=[cc_in.opt()], outs=[cc_out.opt()],
            replica_groups=[list(range(N_CORES))])
        for k in range(NK):
            nc.sync.dma_start(WTfin[:, k, 0, :],
                              cc_out[k].rearrange("(p n) -> p n", p=P))
        nc.sync.dma_start(WTfin[:, 0, 1, :],
                          cc_out[NK].rearrange("(p n) -> p n", p=P))



        # ================= phase 3: weight assembly =================
        PQ_sb = build.tile([P, 4, 2, C], F32R)
        IP_sb = build.tile([P, 4, 2, C], F32R)
        b1T_sb = build.tile([P, 2, 2, 2, C], F32R)
        b2_sb = build.tile([P, 2, 2, 2, C], F32R)
        p3_sb = build.tile([P, 9, 2, C], F32R)

        with tc.tile_pool(name="ps3", bufs=4, space="PSUM") as ps3:
            # Keep the PE drawing steady power through the AllGather wait:
            # the power governor clamps the clock to 13/16 for ~300us when
            # all 8 cores jump from idle to dense matmul at the same moment
            # (post-barrier). ~55us of dummy matmuls bridge the gap; a copy
            # into a live SBUF tile anchors them against DCE.
            warm_sb = small.tile([1, 1], F32, tag="wmsb")
            for wj in range(13):
                warm_ps = ps3.tile([P, 2 * C], F32, tag="wm", bufs=2,
                                   name=f"warm_{wj}")
                for wi in range(20):
                    nc.tensor.matmul(warm_ps[:], WTcur[:, 0, 0:P], flat(WTcur),
                                     start=(wi == 0), stop=(wi == 19))
                nc.vector.tensor_copy(warm_sb[:], warm_ps[0:1, 0:1])
            for i in range(4):
                qps = ps3.tile([P, 2 * C], F32, tag="as", name=f"q_{i}")
                prod_mms(qps, WTfin[:, i + 1], WTfin[:, i + 1], n_tr=1)
                nc.scalar.copy(flat(PQ_sb[:, i]), qps[:])
                nc.vector.tensor_sub(flat(IP_sb[:, i]), flat(I10), flat(PQ_sb[:, i]))

            def symprod(dst3, X3, Y3, nm):
                bps = ps3.tile([P, 2 * C], F32, tag="as", name=f"bp_{nm}")
                prod_mms(bps, X3, Y3)
                nc.scalar.copy(flat(dst3), bps[:])

            for i1 in range(2):
                for j1 in range(2):
                    symprod(b1T_sb[:, i1, j1],
                            IP_sb[:, 1] if j1 else PQ_sb[:, 1],
                            IP_sb[:, 0] if i1 else PQ_sb[:, 0], f"b1T{i1}{j1}")
            for i2 in range(2):
                for j2 in range(2):
                    symprod(b2_sb[:, i2, j2],
                            IP_sb[:, 2] if i2 else PQ_sb[:, 2],
                            IP_sb[:, 3] if j2 else PQ_sb[:, 3], f"b2{i2}{j2}")

            for i in range(3):
                for j in range(3):
                    terms = [(i1, j1) for i1 in range(2) for j1 in range(2)
                             if 0 <= i - i1 < 2 and 0 <= j - j1 < 2]
                    pps = ps3.tile([P, 2 * C], F32, tag="as", name=f"p3_{i}{j}")
                    nmm = len(terms) * 4
                    idx = 0
                    for mt in range(2):
                        for (i1, j1) in terms:
                            for tr in range(2):
                                idx += 1
                                nc.tensor.matmul(
                                    pps[:, mt * C:(mt + 1) * C],
                                    b1T_sb[:, i1, j1, tr, mt * P:(mt + 1) * P],
                                    b2_sb[:, i - i1, j - j1, tr, :],
                                    start=(idx == 1), stop=(idx == nmm))
                    nc.vector.tensor_copy(flat(p3_sb[:, 3 * i + j]), pps[:])

            for i in range(3):
                for j in range(3):
                    eps_ = ps3.tile([P, 2 * C], F32, tag="as", name=f"pe_{i}{j}")
                    prod_mms(eps_, WTfin[:, 0], p3_sb[:, 3 * i + j])
                    nc.scalar.copy(flat(Wc_sb[:, 3 * i + j]), eps_[:])

    # ================= phase 4: conv =================
    with tc.tile_pool(name="opool", bufs=3) as opool, \
         tc.tile_pool(name="psC", bufs=6, space="PSUM") as psC:
        for b in range(B_CORE):
            xp = []
            for tr in range(2):
                xpt = xpool.tile([P, PH, PH], F32R, tag="xp", name=f"xp_{b}_{tr}")
                nc.sync.dma_start(xpt[:, 1:65, 1:65], xs[b, tr * P:(tr + 1) * P, :, :])
                nc.vector.tensor_copy(xpt[:, 1:65, 0:1], xpt[:, 1:65, 64:65])
                nc.vector.tensor_copy(xpt[:, 1:65, 65:66], xpt[:, 1:65, 1:2])
                nc.vector.tensor_copy(xpt[:, 0:1, :], xpt[:, 64:65, :])
                nc.vector.tensor_copy(xpt[:, 65:66, :], xpt[:, 1:2, :])
                xp.append(xpt)
            for mt in range(2):
                osb = opool.tile([P, NPIX], F32, tag="osb", name=f"osb_{b}_{mt}")
                for pb in range(8):
                    ops = psC.tile([P, 512], F32, tag="o")
                    idx = 0
                    for kh in range(3):
                        for kw in range(3):
                            for tr in range(2):
                                nc.tensor.matmul(
                                    ops[:],
                                    Wc_sb[:, 3 * kw + kh, tr, mt * P:(mt + 1) * P],
                                    xp[tr][:, pb * 8 + kh:pb * 8 + kh + 8,
                                           kw:kw + 64],
                                    start=(idx == 0), stop=(idx == 17))
                                idx += 1
                    nc.scalar.activation(
                        osb[:, pb * 512:(pb + 1) * 512], ops[:], ACTF.Identity,
                        bias=bias_sb[:, mt, :], scale=1.0)
                    nc.sync.dma_start(
                        out_ap[b, mt * P:(mt + 1) * P, pb * 8:(pb + 1) * 8, :],
                        osb[:, pb * 512:(pb + 1) * 512].rearrange(
                            "p (h w) -> p h w", w=H))


def build_program():
    from contextlib import ExitStack
    nc = bacc.Bacc("TRN2", target_bir_lowering=False, debug=False,
                   enable_asserts=False, num_devices=N_CORES)
    xs = nc.dram_tensor("xs", [B_CORE, C, H, H], F32R, kind="ExternalInput").ap()
    pmk = nc.dram_tensor("pmk", [C, C], F32R, kind="ExternalInput").ap()
    u0k = nc.dram_tensor("u0k", [C, 1], F32R, kind="ExternalInput").ap()
    bias = nc.dram_tensor("bias", [C], F32, kind="ExternalInput").ap()
    out = nc.dram_tensor("out", [B_CORE, C, H, H], F32, kind="ExternalOutput").ap()
    with tile.TileContext(nc) as tc:
        with ExitStack() as ctx:
            build_body(tc, out, xs, pmk, u0k, bias, ctx)
    nc.compile()
    return nc


_cached_nc = None


def make_in_maps(x, pm, u0, b):
    in_maps = []
    for i in range(N_CORES):
        k = i if i < NK else i - NK
        in_maps.append({
            "xs": np.ascontiguousarray(x[i * B_CORE:(i + 1) * B_CORE]),
            "pmk": np.ascontiguousarray(pm[k]),
            "u0k": np.ascontiguousarray(u0[k]),
            "bias": np.ascontiguousarray(b),
        })
    return in_maps


def kernel(x, param_matrices, init_u, bias):
    global _cached_nc
    if _cached_nc is None:
        _cached_nc = build_program()
    nc = _cached_nc
    x = np.ascontiguousarray(np.asarray(x, dtype=np.float32))
    pm = np.ascontiguousarray(np.asarray(param_matrices, dtype=np.float32))
    u0 = np.ascontiguousarray(np.asarray(init_u, dtype=np.float32))
    b = np.ascontiguousarray(np.asarray(bias, dtype=np.float32))
    in_maps = make_in_maps(x, pm, u0, b)
    res = run_bass_kernel_spmd(nc, in_maps, core_ids=list(range(N_CORES)))
    return np.concatenate([r["out"] for r in res.results], axis=0)


if __name__ == "__main__":
    import reference
    inputs = {k: np.asarray(v) for k, v in reference.setup_inputs().items()}
    out = kernel(**inputs)
    print(out.shape, out.dtype)


# revision 17
# speedup vs baseline: 1.3861x; 1.0311x over previous
"""BCOP (block-convolution orthogonal parameterization) forward on 8 TRN2 cores.

Math (validated vs reference in fp32 numpy):
  - power iteration via repeated squaring: G = A^T A; with v1 = A^T u0,
    d0 = v1.(G^18 v1), d1 = v1.(G^19 v1) reproduce the reference's
    normalized-power-iteration sigma: s = sqrt(d1/d0); G^18 v1 = G16@(G2@v1).
  - W0 = A/s; 20 Bjorck iters maintaining W and WT = W^T:
      G = W^T W  (lhsT=W,rhs=W);  M = 1.5 I - 0.5 G  (symmetric)
      W' = W M   (lhsT=WT,rhs=M); WT' = M WT (lhsT=M,rhs=WT)
  - downstream needs only WT = ortho^T:
      Z^T_i = WT[i+1] with rows>=128 zeroed -> PQ_i = matmul(lhsT=Z^T,rhs=Z^T)
      b1T[i1,j1] = (block_orth(PQ0,PQ1)[i1,j1])^T and b2 = block_orth(PQ2,PQ3),
      all products of symmetric matrices -> no transposes needed.
      p3[i,j] = sum b1[i1,j1] @ b2[i-i1,j-j1] = matmul(lhsT=b1T[..], rhs=b2[..])
      p_e[i,j] = H @ p3[i,j] = matmul(lhsT=WT[0], rhs=p3[i,j])
  - conv tap (kh,kw) uses stationary lhsT[ci,co] = p_e[kw,kh]; x circularly
    padded to 66x66 in SBUF; 18 accumulating matmuls (9 taps x 2 ci-tiles)
    per [128co x 512px] PSUM tile; bias added on the PSUM->SBUF evacuation.

Parallelization: the per-matrix weight construction (power iteration + Bjorck)
is INDEPENDENT across the 5 parameter matrices, so it is sharded across cores
via the inputs (core i receives matrix i mod 5 only), then an 8-rank AllGather
shares the 5 WT results; assembly + conv then run replicated. The conv is
data-parallel over batch (4 images per core).

PSUM discipline: every accumulation group owns a whole bank; a [128,512] tile
holds both 128-row output halves of a 256x256 product as ONE group.
"""

import numpy as np

import concourse.bass as bass
import concourse.mybir as mybir
import concourse.tile as tile
from concourse import bacc
from concourse.bass_utils import run_bass_kernel_spmd

P = 128
C = 256
NK = 5
N_CORES = 8
B_TOTAL = 32
B_CORE = B_TOTAL // N_CORES
H = 64
PH = 66
NPIX = H * H
BJORCK_ITERS = 20

F32 = mybir.dt.float32
F32R = mybir.dt.float32r
ALU = mybir.AluOpType
ACTF = mybir.ActivationFunctionType


def build_body(tc, out_ap, xs, pmk, u0k, bias_ap, ctx):
    nc = tc.nc
    from concourse.masks import make_identity

    persist = ctx.enter_context(tc.tile_pool(name="persist", bufs=1))
    small = ctx.enter_context(tc.tile_pool(name="small", bufs=3))

    U_sb = persist.tile([P, 2, 1], F32R)
    for tr in range(2):
        nc.sync.dma_start(U_sb[:, tr, :], u0k[tr * P:(tr + 1) * P, :])
    bias_sb = persist.tile([P, 2, 1], F32)
    for mt in range(2):
        nc.sync.dma_start(bias_sb[:, mt, :], bias_ap[mt * P:(mt + 1) * P].unsqueeze(1))

    # ---- constants ----
    ID1 = persist.tile([P, P], F32)
    make_identity(nc, ID1)
    I15 = persist.tile([P, 2, C], F32)
    I10 = persist.tile([P, 2, C], F32)
    nc.vector.memset(I15[:], 0.0)
    nc.vector.memset(I10[:], 0.0)
    for mt in range(2):
        nc.scalar.mul(I15[:, mt, mt * P:(mt + 1) * P], ID1[:], 1.5)
        nc.scalar.mul(I10[:, mt, mt * P:(mt + 1) * P], ID1[:], 1.0)
    I15f = I15.rearrange("p a b -> p (a b)")

    RB = persist.tile([P, 1], F32)            # broadcast 1/s (own matrix)
    Wc_sb = persist.tile([P, 9, 2, C], F32R)  # final conv lhsT tiles, slot (i,j)
    WTfin = persist.tile([P, NK, 2, C], F32R)  # gathered ortho^T, all matrices

    def flat(ap3):
        return ap3.rearrange("p a b -> p (a b)")

    def prod_mms(out_ps, X3, Y3, n_tr=2):
        """dst[mt] += X[tr][:, mt]^T @ Y[tr]; ONE accumulation group per bank."""
        first = True
        for mt in range(2):
            for tr in range(n_tr):
                last = (mt == 1 and tr == n_tr - 1)
                nc.tensor.matmul(out_ps[:, mt * C:(mt + 1) * C],
                                 X3[:, tr, mt * P:(mt + 1) * P], Y3[:, tr, :],
                                 start=first, stop=last)
                first = False

    xpool = ctx.enter_context(tc.tile_pool(name="xpool", bufs=4))

    with tc.tile_pool(name="build", bufs=1) as build, \
         tc.tile_pool(name="wstate", bufs=2) as wpool, \
         tc.tile_pool(name="mpool", bufs=4) as mpool, \
         tc.tile_pool(name="vpool", bufs=8) as vpool, \
         tc.tile_pool(name="ccdram", bufs=1, space="DRAM") as ccdram:

        Wcur = wpool.tile([P, 2, C], F32R, tag="W")
        WTcur = wpool.tile([P, 2, C], F32R, tag="WT")
        G_sb = build.tile([P, 2, C], F32R)
        G2_sb = build.tile([P, 2, C], F32R)
        G16_sb = build.tile([P, 2, C], F32R)

        for tr in range(2):
            nc.sync.dma_start(Wcur[:, tr, :], pmk[tr * P:(tr + 1) * P, :])

        # ============ phase 1: sigma via repeated squaring (own matrix) ======
        with tc.tile_pool(name="ps1", bufs=2, space="PSUM") as ps1:
            gps = ps1.tile([P, 2 * C], F32, tag="sq")
            prod_mms(gps, Wcur, Wcur)
            nc.scalar.copy(flat(G_sb), gps[:])

            prev = G_sb
            for pw in (2, 4, 8, 16):
                sq = ps1.tile([P, 2 * C], F32, tag="sq", name=f"sq{pw}")
                prod_mms(sq, prev, prev)
                if pw == 2:
                    dst = G2_sb
                elif pw == 16:
                    dst = G16_sb
                else:
                    dst = build.tile([P, 2, C], F32R, tag="gtmp",
                                     name=f"g{pw}", bufs=2)
                if pw in (4, 16):
                    nc.vector.tensor_copy(flat(dst), sq[:])
                else:
                    nc.scalar.copy(flat(dst), sq[:])
                prev = dst

            def matvec(G3, vin, nm):
                vout = vpool.tile([P, 2], F32R, tag="v", name=f"v_{nm}")
                for mt in range(2):
                    vps = ps1.tile([P, 1], F32, tag="vps", bufs=4,
                                   name=f"vp_{nm}_{mt}")
                    for tr in range(2):
                        nc.tensor.matmul(
                            vps[:], G3[:, tr, mt * P:(mt + 1) * P].bitcast(F32),
                            vin[:, tr:tr + 1].bitcast(F32),
                            start=(tr == 0), stop=(tr == 1))
                    nc.scalar.copy(vout[:, mt:mt + 1], vps[:])
                return vout

            v1 = matvec(Wcur, U_sb, "v1")
            m1 = matvec(G2_sb, v1, "m1")
            m2 = matvec(G16_sb, m1, "m2")
            m3 = matvec(G_sb, m2, "m3")

            def dot(va, vb, nm):
                dps = ps1.tile([1, 1], F32, tag="vps", bufs=4, name=f"d_{nm}")
                for tr in range(2):
                    nc.tensor.matmul(dps[:], va[:, tr:tr + 1].bitcast(F32),
                                     vb[:, tr:tr + 1].bitcast(F32),
                                     start=(tr == 0), stop=(tr == 1))
                return dps

            dps0 = dot(v1, m2, "0")
            dps1 = dot(v1, m3, "1")
            dsb = small.tile([1, 3], F32, tag="dsb")
            nc.vector.tensor_copy(dsb[:, 0:1], dps0[:])
            nc.vector.reciprocal(dsb[:, 1:2], dps1[:])
            nc.vector.tensor_mul(dsb[:, 2:3], dsb[:, 0:1], dsb[:, 1:2])
            rsb = small.tile([1, 1], F32, tag="rsb")
            nc.scalar.sqrt(rsb[:], dsb[:, 2:3])
            nc.gpsimd.partition_broadcast(RB[:, 0:1], rsb[:])
            # W0 = A * r (in place), then WT0 = W0^T via PE transpose
            nc.vector.tensor_scalar_mul(Wcur[:], Wcur[:], RB[:, 0:1])
            for tr in range(2):
                for mt in range(2):
                    tps = ps1.tile([P, P], F32, tag="tp")
                    nc.tensor.transpose(
                        tps[:], Wcur[:, tr, mt * P:(mt + 1) * P].bitcast(F32),
                        ID1[:])
                    nc.scalar.copy(WTcur[:, mt, tr * P:(tr + 1) * P], tps[:])

        # ================= phase 2: Bjorck (own matrix) =================
        with tc.tile_pool(name="ps2", bufs=2, space="PSUM") as ps2:
            for it in range(BJORCK_ITERS):
                last = it == BJORCK_ITERS - 1
                Wnxt = None if last else wpool.tile([P, 2, C], F32R, tag="W",
                                                    name=f"W_{it}")
                WTnxt = wpool.tile([P, 2, C], F32R, tag="WT", name=f"WT_{it}")
                gps = ps2.tile([P, 2 * C], F32, tag="g", bufs=2)
                prod_mms(gps, Wcur, Wcur)
                m_sb = mpool.tile([P, 2 * C], F32R, tag="m", name=f"m_{it}")
                nc.vector.scalar_tensor_tensor(
                    m_sb[:], gps[:], -0.5, I15f, op0=ALU.mult, op1=ALU.add)
                m3 = m_sb.rearrange("p (a b) -> p a b", b=C)
                if not last:
                    wps = ps2.tile([P, 2 * C], F32, tag="w", bufs=2)
                    prod_mms(wps, WTcur, m3)
                    nc.scalar.copy(flat(Wnxt), wps[:])
                wtps = ps2.tile([P, 2 * C], F32, tag="wt", bufs=2)
                prod_mms(wtps, m3, WTcur)
                nc.vector.tensor_copy(flat(WTnxt), wtps[:])
                if Wnxt is not None:
                    Wcur = Wnxt
                WTcur = WTnxt

        # ============ AllGather the needed WT halves across cores ============
        # Assembly reads only row-tile 0 of WT[1..4] (the masked projections)
        # but both row-tiles of WT[0] (H). Cores 0-4 contribute their tr=0
        # half; core 5 (a k=0 duplicate) contributes k=0's tr=1 half via a
        # partition-id-predicated DMA. Halves the AllGather payload.
        cc_in = ccdram.tile([1, P * C], F32R)
        cc_out = ccdram.tile([N_CORES, P * C], F32R, addr_space="Shared")
        pid = nc.sync.partition_id()
        nc.sync.dma_start(cc_in[0].rearrange("(p n) -> p n", p=P),
                          WTcur[:, 0, :], cond=(pid != 5))
        nc.sync.dma_start(cc_in[0].rearrange("(p n) -> p n", p=P),
                          WTcur[:, 1, :], cond=(pid == 5))
        nc.gpsimd.collective_compute(
            "AllGather", ALU.bypass, ins=[cc_in.opt()], outs=[cc_out.opt()],
            replica_groups=[list(range(N_CORES))])
        for k in range(NK):
            nc.sync.dma_start(WTfin[:, k, 0, :],
                              cc_out[k].rearrange("(p n) -> p n", p=P))
        nc.sync.dma_start(WTfin[:, 0, 1, :],
                          cc_out[NK].rearrange("(p n) -> p n", p=P))



        # ================= phase 3: weight assembly =================
        PQ_sb = build.tile([P, 4, 2, C], F32R)
        IP_sb = build.tile([P, 4, 2, C], F32R)
        b1T_sb = build.tile([P, 2, 2, 2, C], F32R)
        b2_sb = build.tile([P, 2, 2, 2, C], F32R)
        p3_sb = build.tile([P, 9, 2, C], F32R)

        with tc.tile_pool(name="ps3", bufs=4, space="PSUM") as ps3:
            for i in range(4):
                qps = ps3.tile([P, 2 * C], F32, tag="as", name=f"q_{i}")
                prod_mms(qps, WTfin[:, i + 1], WTfin[:, i + 1], n_tr=1)
                nc.scalar.copy(flat(PQ_sb[:, i]), qps[:])
                nc.vector.tensor_sub(flat(IP_sb[:, i]), flat(I10), flat(PQ_sb[:, i]))

            def symprod(dst3, X3, Y3, nm):
                bps = ps3.tile([P, 2 * C], F32, tag="as", name=f"bp_{nm}")
                prod_mms(bps, X3, Y3)
                nc.scalar.copy(flat(dst3), bps[:])

            for i1 in range(2):
                for j1 in range(2):
                    symprod(b1T_sb[:, i1, j1],
                            IP_sb[:, 1] if j1 else PQ_sb[:, 1],
                            IP_sb[:, 0] if i1 else PQ_sb[:, 0], f"b1T{i1}{j1}")
            for i2 in range(2):
                for j2 in range(2):
                    symprod(b2_sb[:, i2, j2],
                            IP_sb[:, 2] if i2 else PQ_sb[:, 2],
                            IP_sb[:, 3] if j2 else PQ_sb[:, 3], f"b2{i2}{j2}")

            for i in range(3):
                for j in range(3):
                    terms = [(i1, j1) for i1 in range(2) for j1 in range(2)
                             if 0 <= i - i1 < 2 and 0 <= j - j1 < 2]
                    pps = ps3.tile([P, 2 * C], F32, tag="as", name=f"p3_{i}{j}")
                    nmm = len(terms) * 4
                    idx = 0
                    for mt in range(2):
                        for (i1, j1) in terms:
                            for tr in range(2):
                                idx += 1
                                nc.tensor.matmul(
                                    pps[:, mt * C:(mt + 1) * C],
                                    b1T_sb[:, i1, j1, tr, mt * P:(mt + 1) * P],
                                    b2_sb[:, i - i1, j - j1, tr, :],
                                    start=(idx == 1), stop=(idx == nmm))
                    nc.vector.tensor_copy(flat(p3_sb[:, 3 * i + j]), pps[:])

            for i in range(3):
                for j in range(3):
                    eps_ = ps3.tile([P, 2 * C], F32, tag="as", name=f"pe_{i}{j}")
                    prod_mms(eps_, WTfin[:, 0], p3_sb[:, 3 * i + j])
                    nc.scalar.copy(flat(Wc_sb[:, 3 * i + j]), eps_[:])

    # ================= phase 4: conv =================
    with tc.tile_pool(name="opool", bufs=3) as opool, \
         tc.tile_pool(name="psC", bufs=6, space="PSUM") as psC:
        for b in range(B_CORE):
            xp = []
            for tr in range(2):
                xpt = xpool.tile([P, PH, PH], F32R, tag="xp", name=f"xp_{b}_{tr}")
                nc.sync.dma_start(xpt[:, 1:65, 1:65], xs[b, tr * P:(tr + 1) * P, :, :])
                nc.vector.tensor_copy(xpt[:, 1:65, 0:1], xpt[:, 1:65, 64:65])
                nc.vector.tensor_copy(xpt[:, 1:65, 65:66], xpt[:, 1:65, 1:2])
                nc.vector.tensor_copy(xpt[:, 0:1, :], xpt[:, 64:65, :])
                nc.vector.tensor_copy(xpt[:, 65:66, :], xpt[:, 1:2, :])
                xp.append(xpt)
            for mt in range(2):
                osb = opool.tile([P, NPIX], F32, tag="osb", name=f"osb_{b}_{mt}")
                for pb in range(8):
                    ops = psC.tile([P, 512], F32, tag="o")
                    idx = 0
                    for kh in range(3):
                        for kw in range(3):
                            for tr in range(2):
                                nc.tensor.matmul(
                                    ops[:],
                                    Wc_sb[:, 3 * kw + kh, tr, mt * P:(mt + 1) * P],
                                    xp[tr][:, pb * 8 + kh:pb * 8 + kh + 8,
                                           kw:kw + 64],
                                    start=(idx == 0), stop=(idx == 17))
                                idx += 1
                    nc.scalar.activation(
                        osb[:, pb * 512:(pb + 1) * 512], ops[:], ACTF.Identity,
                        bias=bias_sb[:, mt, :], scale=1.0)
                    nc.sync.dma_start(
                        out_ap[b, mt * P:(mt + 1) * P, pb * 8:(pb + 1) * 8, :],
                        osb[:, pb * 512:(pb + 1) * 512].rearrange(
                            "p (h w) -> p h w", w=H))


def build_program():
    from contextlib import ExitStack
    nc = bacc.Bacc("TRN2", target_bir_lowering=False, debug=False,
                   enable_asserts=False, num_devices=N_CORES)
    xs = nc.dram_tensor("xs", [B_CORE, C, H, H], F32R, kind="ExternalInput").ap()
    pmk = nc.dram_tensor("pmk", [C, C], F32R, kind="ExternalInput").ap()
    u0k = nc.dram_tensor("u0k", [C, 1], F32R, kind="ExternalInput").ap()
    bias = nc.dram_tensor("bias", [C], F32, kind="ExternalInput").ap()
    out = nc.dram_tensor("out", [B_CORE, C, H, H], F32, kind="ExternalOutput").ap()
    with tile.TileContext(nc) as tc:
        with ExitStack() as ctx:
            build_body(tc, out, xs, pmk, u0k, bias, ctx)
    nc.compile()
    return nc


_cached_nc = None


def make_in_maps(x, pm, u0, b):
    in_maps = []
    for i in range(N_CORES):
        k = i if i < NK else i - NK
        in_maps.append({
            "xs": np.ascontiguousarray(x[i * B_CORE:(i + 1) * B_CORE]),
            "pmk": np.ascontiguousarray(pm[k]),
            "u0k": np.ascontiguousarray(u0[k]),
            "bias": np.ascontiguousarray(b),
        })
    return in_maps


def kernel(x, param_matrices, init_u, bias):
    global _cached_nc
    if _cached_nc is None:
        _cached_nc = build_program()
    nc = _cached_nc
    x = np.ascontiguousarray(np.asarray(x, dtype=np.float32))
    pm = np.ascontiguousarray(np.asarray(param_matrices, dtype=np.float32))
    u0 = np.ascontiguousarray(np.asarray(init_u, dtype=np.float32))
    b = np.ascontiguousarray(np.asarray(bias, dtype=np.float32))
    in_maps = make_in_maps(x, pm, u0, b)
    res = run_bass_kernel_spmd(nc, in_maps, core_ids=list(range(N_CORES)))
    return np.concatenate([r["out"] for r in res.results], axis=0)


if __name__ == "__main__":
    import reference
    inputs = {k: np.asarray(v) for k, v in reference.setup_inputs().items()}
    out = kernel(**inputs)
    print(out.shape, out.dtype)
